# revision 1
# baseline (speedup 1.0000x reference)
"""Causal self-attention + residual + LayerNorm fused Trainium2 kernel.

Problem: B=4, S=2048, D=1024, H=16 heads (hd=64), fp32 in/out.
    qkv = x @ in_proj_w.T + in_proj_b ; causal MHA ; out proj ; y = LN(x + attn_out)

Sharding (zero cross-core communication, 8 NeuronCores):
    core c -> batch b = c % 4, query-group g = c // 4.
    Causal zig-zag balance: g=0 owns query blocks [0:512) and [1536:2048),
    g=1 owns [512:1536). Every core computes full K/V for its batch
    (keys 0:2048), attention only for its own queries, then out-proj +
    residual + LayerNorm for its queries. Outputs are disjoint row sets.

Layout: everything is computed transposed (features on partitions,
tokens on the free axis), which makes every matmul contraction land on
the partition axis with zero on-chip transposes:
    K^T[f,t] / Q^T[f,q] = W^T-tile.T @ x^T        (lhsT = in_proj_w.T tile)
    V[t,f]              = x^T-tile.T @ W^T        (lhsT = x^T tile)
    S^T[k,q]            = K^T-slice.T @ Q^T-slice (contraction = head dim 64,
                                                   two heads packed in the PE
                                                   array via tile_position)
    ctx^T[d,q]          = V-slice.T @ exp(S^T)    (V augmented with a ones
                                                   column -> row 64 of the
                                                   PSUM tile = softmax denom)
    out^T[Do,q]         = out_w.T-tile.T @ ctx^T
    LN stats            = ones.T @ y / ones.T @ y^2 (partition reduction on PE)
Matmuls run in float32r (TF32-like, ~11-bit mantissa, 4x faster than fp32
on the PE; measured end-to-end relerr ~1.5e-4). Softmax skips the max
subtraction (scores ~ N(0,1)) and defers the divide: ctx is normalized by
the reciprocal of the aug-row denominator, broadcast across partitions by
the GPSIMD partition_broadcast op.

The two query-groups differ only in the per-q-tile causal k-tile counts;
both variants are emitted under a tc.If on the partition id, so one SPMD
program serves all 8 cores in a single launch.
"""
import sys

if "/opt/trn_rl_repo" not in sys.path:
    sys.path.insert(0, "/opt/trn_rl_repo")

import numpy as np

B, S, D, H, HD = 4, 2048, 1024, 16, 64
P = 128
QT = 512                      # queries per q-tile (matmul free dim)
NQ = 1024                     # queries per core
NKT = S // P                  # 16 k-tiles per batch
DK = D // P                   # 8 contraction tiles over D
NPLAIN = {0: (0, 12), 1: (4, 8)}   # group -> per-q-tile plain (unmasked) k-tiles

_cache = {}


def _build():
    import concourse.mybir as mybir
    import concourse.tile as tile
    from concourse import bacc
    from concourse.bass import ts
    from concourse.alu_op_type import AluOpType

    f32 = mybir.dt.float32
    f32r = mybir.dt.float32r
    AF = mybir.ActivationFunctionType

    nc = bacc.Bacc("TRN2", target_bir_lowering=False, debug=False, num_devices=8)

    xkv = nc.dram_tensor("xkv", [D, S], f32r, kind="ExternalInput").ap()
    xq = nc.dram_tensor("xq", [D, NQ], f32r, kind="ExternalInput").ap()
    wt = nc.dram_tensor("wt", [D, 3 * D], f32r, kind="ExternalInput").ap()
    wot = nc.dram_tensor("wot", [D, D], f32r, kind="ExternalInput").ap()
    maskd = nc.dram_tensor("maskd", [P, 896], f32r, kind="ExternalInput").ap()
    bqd = nc.dram_tensor("bqd", [D], f32, kind="ExternalInput").ap()
    bkd = nc.dram_tensor("bkd", [D], f32, kind="ExternalInput").ap()
    bvd = nc.dram_tensor("bvd", [D], f32, kind="ExternalInput").ap()
    bod = nc.dram_tensor("bod", [D], f32, kind="ExternalInput").ap()
    gamd = nc.dram_tensor("gamd", [D], f32, kind="ExternalInput").ap()
    betd = nc.dram_tensor("betd", [D], f32, kind="ExternalInput").ap()
    yt = nc.dram_tensor("yt", [D, NQ], f32, kind="ExternalOutput").ap()

    xkv_r = xkv.rearrange("(dk p) t -> p dk t", p=P)
    xq_r = xq.rearrange("(dk p) q -> p dk q", p=P)
    xq_f32 = xq.bitcast(f32).rearrange("(ok p) q -> p ok q", p=P)

    with tile.TileContext(nc) as tc:
        with (
            tc.tile_pool(name="persist", bufs=1) as pers,
            tc.tile_pool(name="proj_ps", bufs=2, space="PSUM") as proj_ps,
        ):
            kt = pers.tile([P, DK, S], f32r)           # K^T       64 KB/part
            msk = pers.tile([P, 896], f32r)            #           3.5 KB
            bia = pers.tile([P, DK, 6], f32)           # bq bk bv bo gam bet
            ones128 = pers.tile([P, 1], f32r)
            eps_t = pers.tile([1, 1], f32)
            nc.vector.memset(eps_t[:], 1e-5)

            nc.sync.dma_start(msk[:], maskd[:])
            for j, src in enumerate((bqd, bkd, bvd, bod, gamd, betd)):
                nc.sync.dma_start(bia[:, :, j], src.rearrange("(f p) -> p f", p=P))
            nc.vector.memset(ones128[:].bitcast(f32), 1.0)

            def bq_(f): return bia[:, f, 0:1]
            def bk_(f): return bia[:, f, 1:2]
            def bo_(f): return bia[:, f, 3:4]
            def gam_(f): return bia[:, f, 4:5]
            def bet_(f): return bia[:, f, 5:6]

            # ---- phase A: K^T projection ------------------------------
            with (
                tc.tile_pool(name="wk", bufs=1) as wkp,
                tc.tile_pool(name="xa", bufs=2) as xap,
            ):
                wk = wkp.tile([P, DK, DK, P], f32r)
                nc.sync.dma_start(
                    wk[:],
                    wt[:, D:2 * D].rearrange("(dk p) (f c) -> p dk f c", p=P, c=P),
                )
                for t in range(S // QT):
                    xc = xap.tile([P, DK, QT], f32r, tag="xa")
                    nc.sync.dma_start(xc[:], xkv_r[:, :, ts(t, QT)])
                    for f in range(DK):
                        ps = proj_ps.tile([P, QT], f32, tag="pp")
                        for dk in range(DK):
                            nc.tensor.matmul(
                                ps[:], wk[:, dk, f, :], xc[:, dk, :],
                                start=(dk == 0), stop=(dk == DK - 1),
                            )
                        nc.vector.tensor_scalar_add(kt[:, f, ts(t, QT)], ps[:], bk_(f))

            with tc.tile_pool(name="vpool", bufs=1) as vp:
                v = vp.tile([P, NKT, H, HD + 1], f32r)   # V aug  65 KB/part
                nc.vector.memset(v[:, :, :, HD].bitcast(f32), 1.0)

                # ---- phase B: V projection (natural orientation) ------
                with (
                    tc.tile_pool(name="wv", bufs=1) as wvp,
                    tc.tile_pool(name="xb", bufs=4) as xbp,
                ):
                    wv = wvp.tile([P, DK, 2, 512], f32r)
                    nc.sync.dma_start(
                        wv[:],
                        wt[:, 2 * D:3 * D].rearrange(
                            "(dk p) (g c) -> p dk g c", p=P, c=512),
                    )
                    for t in range(NKT):
                        xc = xbp.tile([P, DK, P], f32r, tag="xb")
                        nc.sync.dma_start(xc[:], xkv_r[:, :, ts(t, P)])
                        for fg in range(2):
                            ps = proj_ps.tile([P, 512], f32, tag="pp")
                            for dk in range(DK):
                                nc.tensor.matmul(
                                    ps[:], xc[:, dk, :], wv[:, dk, fg, :],
                                    start=(dk == 0), stop=(dk == DK - 1),
                                )
                            for hh in range(8):
                                h = 8 * fg + hh
                                nc.vector.tensor_copy(
                                    v[:, t, h, 0:HD], ps[:, ts(hh, HD)]
                                )

                # ---- phases C-F under the partition-id branch ---------
                with tc.tile_pool(name="qc", bufs=1) as qcp:
                    ctx = qcp.tile([P, DK, QT], f32r)

                    def qproj(qt, qtile):
                        with (
                            tc.tile_pool(name="wq", bufs=2) as wqp,
                            tc.tile_pool(name="xqp", bufs=1) as xqp,
                        ):
                            xc = xqp.tile([P, DK, QT], f32r, tag="xq")
                            nc.sync.dma_start(xc[:], xq_r[:, :, ts(qt, QT)])
                            for f in range(DK):
                                wq = wqp.tile([P, DK, P], f32r, tag="wq")
                                nc.sync.dma_start(
                                    wq[:],
                                    wt[:, ts(f, P)].rearrange(
                                        "(dk p) c -> p dk c", p=P),
                                )
                                ps = proj_ps.tile([P, QT], f32, tag="pp")
                                for dk in range(DK):
                                    nc.tensor.matmul(
                                        ps[:], wq[:, dk, :], xc[:, dk, :],
                                        start=(dk == 0), stop=(dk == DK - 1),
                                    )
                                nc.vector.tensor_scalar_add(
                                    qtile[:, f, :], ps[:], bq_(f))

                    def attn(n_plain, qtile):
                        nk = n_plain + 4
                        with (
                            tc.tile_pool(name="sep", bufs=4) as sep,
                            tc.tile_pool(name="scr", bufs=2) as scr,
                            tc.tile_pool(name="s_ps", bufs=2, space="PSUM") as s_ps,
                            tc.tile_pool(name="c_ps", bufs=2, space="PSUM") as c_ps,
                        ):
                            for hp in range(H // 2):
                                cp0 = c_ps.tile([HD + 1, QT], f32, tag="c0")
                                cp1 = c_ps.tile([HD + 1, QT], f32, tag="c1")
                                for i in range(nk):
                                    sp0 = s_ps.tile([P, QT], f32, tag="s")
                                    sp1 = s_ps.tile([P, QT], f32, tag="s")
                                    nc.tensor.matmul(
                                        sp0[:], kt[0:HD, hp, ts(i, P)],
                                        qtile[0:HD, hp, :], start=True, stop=True,
                                    )
                                    nc.tensor.matmul(
                                        sp1[:], kt[HD:P, hp, ts(i, P)],
                                        qtile[HD:P, hp, :], start=True, stop=True,
                                    )
                                    se0 = sep.tile([P, QT], f32r, tag="se")
                                    se1 = sep.tile([P, QT], f32r, tag="se")
                                    nc.scalar.activation(
                                        se0[:], sp0[:], AF.Exp, scale=0.125)
                                    nc.scalar.activation(
                                        se1[:], sp1[:], AF.Exp, scale=0.125)
                                    if i >= n_plain:
                                        off = 384 - P * (i - n_plain)
                                        nc.vector.tensor_mul(
                                            se0[:], se0[:], msk[:, off:off + QT])
                                        nc.vector.tensor_mul(
                                            se1[:], se1[:], msk[:, off:off + QT])
                                    nc.tensor.matmul(
                                        cp0[:], v[:, i, 2 * hp, :], se0[:],
                                        start=(i == 0), stop=(i == nk - 1),
                                    )
                                    nc.tensor.matmul(
                                        cp1[:], v[:, i, 2 * hp + 1, :], se1[:],
                                        start=(i == 0), stop=(i == nk - 1),
                                    )
                                for j, cp in ((0, cp0), (1, cp1)):
                                    h = 2 * hp + j
                                    po, ft = HD * (h % 2), h // 2
                                    den = scr.tile([1, QT], f32, tag="den")
                                    nc.vector.tensor_copy(den[:], cp[HD:HD + 1, :])
                                    rec = scr.tile([1, QT], f32, tag="rec")
                                    rscr = scr.tile([1, QT], f32, tag="rscr")
                                    nc.vector.reciprocal_approx_accurate(
                                        rec[:], den[:], rscr[:])
                                    bc = scr.tile([HD, QT], f32, tag="bc")
                                    nc.gpsimd.partition_broadcast(bc[:], rec[:])
                                    dst = ctx[po:po + HD, ft, :]
                                    nc.vector.tensor_mul(dst, cp[0:HD, :], bc[:])
                                    nc.vector.tensor_scalar_add(
                                        dst, dst, bia[po:po + HD, ft, 2:3])

                    def outproj_ln(qt):
                        with (
                            tc.tile_pool(name="wo", bufs=3) as wop,
                            tc.tile_pool(name="ep", bufs=1) as ep,
                            tc.tile_pool(name="st_ps", bufs=2, space="PSUM") as st_ps,
                        ):
                            y = ep.tile([P, DK, QT], f32r, tag="y")
                            for o in range(DK):
                                wo = wop.tile([P, DK, P], f32r, tag="wo")
                                nc.sync.dma_start(
                                    wo[:],
                                    wot[:, ts(o, P)].rearrange(
                                        "(dk p) c -> p dk c", p=P),
                                )
                                ps = proj_ps.tile([P, QT], f32, tag="pp")
                                for dk in range(DK):
                                    nc.tensor.matmul(
                                        ps[:], wo[:, dk, :], ctx[:, dk, :],
                                        start=(dk == 0), stop=(dk == DK - 1),
                                    )
                                xr = ep.tile([P, QT], f32, tag="xr", bufs=3)
                                nc.sync.dma_start(xr[:], xq_f32[:, o, ts(qt, QT)])
                                nc.vector.scalar_tensor_tensor(
                                    y[:, o, :], ps[:], bo_(o), xr[:],
                                    AluOpType.add, AluOpType.add,
                                )
                            mu_ps = st_ps.tile([1, QT], f32, tag="mu")
                            for o in range(DK):
                                nc.tensor.matmul(
                                    mu_ps[:], ones128[:], y[:, o, :],
                                    start=(o == 0), stop=(o == DK - 1))
                            ms_ps = st_ps.tile([1, QT], f32, tag="ms")
                            for o in range(DK):
                                ysq = ep.tile([P, QT], f32r, tag="ysq")
                                nc.vector.tensor_mul(
                                    ysq[:], y[:, o, :], y[:, o, :])
                                nc.tensor.matmul(
                                    ms_ps[:], ones128[:], ysq[:],
                                    start=(o == 0), stop=(o == DK - 1))
                            mu = ep.tile([1, QT], f32, tag="mu_sb")
                            nc.scalar.mul(mu[:], mu_ps[:], 1.0 / D)
                            ms = ep.tile([1, QT], f32, tag="ms_sb")
                            nc.scalar.mul(ms[:], ms_ps[:], 1.0 / D)
                            tmp = ep.tile([1, QT], f32, tag="stat_tmp", bufs=2)
                            nc.vector.tensor_mul(tmp[:], mu[:], mu[:])
                            nc.vector.tensor_sub(ms[:], ms[:], tmp[:])  # var
                            sd = ep.tile([1, QT], f32, tag="stat_tmp", bufs=2)
                            nc.scalar.activation(sd[:], ms[:], AF.Sqrt, bias=eps_t[:])
                            rstd = ep.tile([1, QT], f32, tag="rstd")
                            rsc = ep.tile([1, QT], f32, tag="stat_tmp", bufs=2)
                            nc.vector.reciprocal_approx_accurate(
                                rstd[:], sd[:], rsc[:])
                            mu_bc = ep.tile([P, QT], f32, tag="mu_bc")
                            nc.gpsimd.partition_broadcast(mu_bc[:], mu[:])
                            rs_bc = ep.tile([P, QT], f32, tag="rs_bc")
                            nc.gpsimd.partition_broadcast(rs_bc[:], rstd[:])
                            for o in range(DK):
                                t1 = ep.tile([P, QT], f32, tag="t1", bufs=2)
                                nc.vector.tensor_sub(
                                    t1[:], y[:, o, :].bitcast(f32), mu_bc[:])
                                nc.vector.tensor_mul(t1[:], t1[:], rs_bc[:])
                                yo = ep.tile([P, QT], f32, tag="yo", bufs=2)
                                nc.vector.tensor_scalar(
                                    yo[:], t1[:], gam_(o), bet_(o),
                                    AluOpType.mult, AluOpType.add,
                                )
                                nc.sync.dma_start(yt[ts(o, P), ts(qt, QT)], yo[:])

                    def group(g):
                        for qt in range(2):
                            with tc.tile_pool(name="qtp", bufs=1) as qtp:
                                qtile = qtp.tile([P, DK, QT], f32r, tag="qtile")
                                qproj(qt, qtile)
                                attn(NPLAIN[g][qt], qtile)
                            outproj_ln(qt)

                    pid = nc.partition_id()
                    with tc.If(pid < 4) as cmp:
                        group(0)
                    with cmp.Else():
                        group(1)
    nc.compile()
    return nc


def _get_nc():
    if "nc" not in _cache:
        _cache["nc"] = _build()
    return _cache["nc"]


def _prep(x, in_proj_w, in_proj_b, out_w, out_b, gamma, beta):
    x = np.asarray(x, np.float32)
    wt = np.ascontiguousarray(np.asarray(in_proj_w, np.float32).T)
    wot = np.ascontiguousarray(np.asarray(out_w, np.float32).T)
    bqkv = np.asarray(in_proj_b, np.float32)
    bo = np.asarray(out_b, np.float32)
    gam = np.asarray(gamma, np.float32)
    bet = np.asarray(beta, np.float32)
    ku = np.arange(P)[:, None] <= (np.arange(896)[None, :] - 384)
    maskd = ku.astype(np.float32)
    qcols = {
        0: np.r_[0:QT, 3 * QT:4 * QT],
        1: np.r_[QT:3 * QT],
    }
    in_maps = []
    for c in range(8):
        b, g = c % 4, c // 4
        xt = np.ascontiguousarray(x[b].T)
        in_maps.append({
            "xkv": xt,
            "xq": np.ascontiguousarray(xt[:, qcols[g]]),
            "wt": wt,
            "wot": wot,
            "maskd": maskd,
            "bqd": bqkv[0:D], "bkd": bqkv[D:2 * D], "bvd": bqkv[2 * D:3 * D],
            "bod": bo, "gamd": gam, "betd": bet,
        })
    return in_maps, qcols


def _run(in_maps, trace=False, **kw):
    from concourse.bass_utils import run_bass_kernel_spmd

    return run_bass_kernel_spmd(_get_nc(), in_maps, list(range(8)), trace=trace, **kw)


def kernel(x, in_proj_w, in_proj_b, out_w, out_b, gamma, beta):
    in_maps, qcols = _prep(x, in_proj_w, in_proj_b, out_w, out_b, gamma, beta)
    res = _run(in_maps)
    out = np.empty((B, S, D), np.float32)
    for c in range(8):
        out[c % 4, qcols[c // 4]] = res.results[c]["yt"].T
    return out



# revision 2
# speedup vs baseline: 1.7922x; 1.7922x over previous
"""Causal self-attention + residual + LayerNorm fused Trainium2 kernel (v2).

Problem: B=4, S=2048, D=1024, H=16 heads (hd=64), fp32 in/out.
    qkv = x @ in_proj_w.T + in_proj_b ; causal MHA ; out proj ; y = LN(x + attn_out)

Sharding (zero cross-core communication, 8 NeuronCores):
    core c -> batch b = c % 4, query-group g = c // 4.
    Causal zig-zag balance: g=0 owns query blocks [0:512) and [1536:2048),
    g=1 owns [512:1536). Every core computes full K/V for its batch,
    attention only for its own queries, then out-proj + residual +
    LayerNorm for its queries. Outputs are disjoint row sets.

v2 changes vs v1 (baseline 829us):
  * bf16 matmul datapath everywhere (weights, x, K^T, V, Q^T, exp-scores,
    ctx). PE rate is the same as f32r but FWL halves weight-load time,
    DVE ops double their rate, DMA bytes halve. PSUM accumulation stays
    fp32; the residual stream and LN statistics stay fp32-exact enough
    (y in bf16, stats summed in fp32 PSUM).
  * exp merged to [128, 2, 512] PSUM tiles (both heads of a pair in one
    ACTIVATE): 160 instead of 320 activations -> ACT busy 230 -> 183us.
  * x is DMAed once and stays resident in SBUF; Q-projection reads
    column slices of it (no separate xq stream), V-projection reuses it.
  * V-projection PSUM->SBUF moves batched to one strided scalar-engine
    copy per 512-feature block (was 8 DVE copies), freeing the DVE.
  * in_proj v-bias folded into the out-proj bias on the host
    (bo_eff = out_b + out_w @ bv), removing a per-head DVE add.
  * Emission interleaves V-projection / Q-projection(qt1) chunks into
    attention(qt0), and out-projection(qt0) chunks into attention(qt1),
    keeping the PE dense so the HAM clock gate stays at full rate.

Layout: everything transposed (features on partitions, tokens free), so
every matmul contraction lands on the partition axis with zero on-chip
transposes. Softmax skips max-subtraction (scores ~ N(0,1)); the divide
is deferred via a ones-augmented V column (row 64 of the ctx PSUM tile
accumulates the denominator).
"""
import sys

if "/opt/trn_rl_repo" not in sys.path:
    sys.path.insert(0, "/opt/trn_rl_repo")

import numpy as np

B, S, D, H, HD = 4, 2048, 1024, 16, 64
P = 128
QT = 512                      # queries per q-tile (matmul free dim)
NQ = 1024                     # queries per core
NKT = S // P                  # 16 k-tiles per batch
DK = D // P                   # 8 contraction tiles over D
NPLAIN = {0: (0, 12), 1: (4, 8)}   # group -> per-q-tile plain (unmasked) k-tiles
QBLOCK = {0: (0, 3), 1: (1, 2)}    # group -> 512-col x-block per q-tile

_cache = {}


def _build():
    import concourse.mybir as mybir
    import concourse.tile as tile
    from concourse import bacc
    from concourse.bass import ts
    from concourse.alu_op_type import AluOpType

    f32 = mybir.dt.float32
    bf16 = mybir.dt.bfloat16
    AF = mybir.ActivationFunctionType

    nc = bacc.Bacc("TRN2", target_bir_lowering=False, debug=False, num_devices=8)

    xkv = nc.dram_tensor("xkv", [D, S], bf16, kind="ExternalInput").ap()
    xrd = nc.dram_tensor("xrd", [D, S], f32, kind="ExternalInput").ap()
    wqd = nc.dram_tensor("wqd", [D, D], bf16, kind="ExternalInput").ap()
    wkd = nc.dram_tensor("wkd", [D, D], bf16, kind="ExternalInput").ap()
    wvd = nc.dram_tensor("wvd", [D, D], bf16, kind="ExternalInput").ap()
    wod = nc.dram_tensor("wod", [D, D], bf16, kind="ExternalInput").ap()
    maskd = nc.dram_tensor("maskd", [P, 896], bf16, kind="ExternalInput").ap()
    bqd = nc.dram_tensor("bqd", [D], f32, kind="ExternalInput").ap()
    bkd = nc.dram_tensor("bkd", [D], f32, kind="ExternalInput").ap()
    bod = nc.dram_tensor("bod", [D], f32, kind="ExternalInput").ap()
    gamd = nc.dram_tensor("gamd", [D], f32, kind="ExternalInput").ap()
    betd = nc.dram_tensor("betd", [D], f32, kind="ExternalInput").ap()
    yt = nc.dram_tensor("yt", [D, NQ], f32, kind="ExternalOutput").ap()

    xkv_r = xkv.rearrange("(dk p) t -> p dk t", p=P)
    xr_r = xrd.rearrange("(ok p) t -> p ok t", p=P)
    wq_r = wqd.rearrange("(dk p) (f c) -> p dk f c", p=P, c=P)
    wk_r = wkd.rearrange("(dk p) (f c) -> p dk f c", p=P, c=P)
    wv_r = wvd.rearrange("(dk p) (g c) -> p dk g c", p=P, c=512)
    wo_r = wod.rearrange("(dk p) (o c) -> p dk o c", p=P, c=P)

    with tile.TileContext(nc) as tc:
        with tc.tile_pool(name="pers", bufs=1) as pers:
            kt = pers.tile([P, DK, S], bf16)           # K^T        32 KB/part
            v = pers.tile([P, NKT, H, HD + 1], bf16)   # V aug      33.25 KB
            msk = pers.tile([P, 896], bf16)
            bia = pers.tile([P, DK, 5], f32)           # bq bk bo gam bet
            ones128 = pers.tile([P, 1], bf16)
            eps_t = pers.tile([1, 1], f32)
            wo = pers.tile([P, DK, DK, P], bf16)       # out_w^T    16 KB
            qtl = [
                pers.tile([P, DK, QT], bf16, tag="qtl0", name="qtl0"),
                pers.tile([P, DK, QT], bf16, tag="qtl1", name="qtl1"),
            ]
            ctx = [
                pers.tile([P, DK, QT], bf16, tag="ctx0", name="ctx0"),
                pers.tile([P, DK, QT], bf16, tag="ctx1", name="ctx1"),
            ]

            nc.vector.memset(eps_t[:], 1e-5)
            nc.vector.memset(ones128[:], 1.0)
            nc.vector.memset(v[:, :, :, HD], 1.0)
            nc.sync.dma_start(msk[:], maskd[:])
            nc.sync.dma_start(wo[:], wo_r)
            for j, src in enumerate((bqd, bkd, bod, gamd, betd)):
                nc.sync.dma_start(bia[:, :, j], src.rearrange("(f p) -> p f", p=P))

            def bq_(f): return bia[:, f, 0:1]
            def bk_(f): return bia[:, f, 1:2]
            def bo_(f): return bia[:, f, 2:3]
            def gam_(f): return bia[:, f, 3:4]
            def bet_(f): return bia[:, f, 4:5]

            def emit_group(g):
                npl = NPLAIN[g]
                blocks = QBLOCK[g]

                with tc.tile_pool(name=f"pp{g}", bufs=2, space="PSUM") as pp:

                    def qproj(qt):
                        xs = x[:, :, ts(blocks[qt], QT)]
                        for f in range(DK):
                            ps = pp.tile([P, QT], f32, tag="pp", name="psq")
                            for dk in range(DK):
                                nc.tensor.matmul(
                                    ps[:], wqs[:, dk, f, :], xs[:, dk, :],
                                    start=(dk == 0), stop=(dk == DK - 1),
                                )
                            nc.vector.tensor_scalar_add(
                                qtl[qt][:, f, :], ps[:], bq_(f))

                    def bchunk(t):
                        for fg in range(2):
                            ps = pp.tile([P, 8, HD], f32, tag="pp", name="psv")
                            for dk in range(DK):
                                nc.tensor.matmul(
                                    ps[:], x[:, dk, ts(t, P)], wvs[:, dk, fg, :],
                                    start=(dk == 0), stop=(dk == DK - 1),
                                )
                            nc.scalar.copy(v[:, t, 8 * fg:8 * fg + 8, 0:HD], ps[:])

                    def attn(qt, fillers):
                        nk = npl[qt] + 4
                        qtile = qtl[qt]
                        with (
                            tc.tile_pool(name=f"sep{g}{qt}", bufs=4) as sep,
                            tc.tile_pool(name=f"scr{g}{qt}", bufs=1) as scr,
                            tc.tile_pool(name=f"sps{g}{qt}", bufs=2, space="PSUM") as s_ps,
                            tc.tile_pool(name=f"cps{g}{qt}", bufs=1, space="PSUM") as c_ps,
                        ):
                            nf = len(fillers)
                            for hp in range(H // 2):
                                cp0 = c_ps.tile([HD + 1, QT], f32, tag="c0", name="cp0")
                                cp1 = c_ps.tile([HD + 1, QT], f32, tag="c1", name="cp1")
                                for i in range(nk):
                                    sp = s_ps.tile([P, 2, QT], f32, tag="s", name="sp")
                                    se = sep.tile([P, 2, QT], bf16, tag="se", name="se")
                                    nc.tensor.matmul(
                                        sp[:, 0, :], kt[0:HD, hp, ts(i, P)],
                                        qtile[0:HD, hp, :], start=True, stop=True,
                                    )
                                    nc.tensor.matmul(
                                        sp[:, 1, :], kt[HD:P, hp, ts(i, P)],
                                        qtile[HD:P, hp, :], start=True, stop=True,
                                    )
                                    nc.scalar.activation(
                                        se[:], sp[:], AF.Exp, scale=0.125)
                                    if i >= npl[qt]:
                                        off = 384 - P * (i - npl[qt])
                                        nc.vector.tensor_mul(
                                            se[:, 0, :], se[:, 0, :],
                                            msk[:, off:off + QT])
                                        nc.vector.tensor_mul(
                                            se[:, 1, :], se[:, 1, :],
                                            msk[:, off:off + QT])
                                    nc.tensor.matmul(
                                        cp0[:], v[:, i, 2 * hp, :], se[:, 0, :],
                                        start=(i == 0), stop=(i == nk - 1),
                                    )
                                    nc.tensor.matmul(
                                        cp1[:], v[:, i, 2 * hp + 1, :], se[:, 1, :],
                                        start=(i == 0), stop=(i == nk - 1),
                                    )
                                for j, cp in ((0, cp0), (1, cp1)):
                                    h = 2 * hp + j
                                    po, ft = HD * (h % 2), h // 2
                                    den = scr.tile([1, QT], f32, tag="den", bufs=2)
                                    nc.vector.tensor_copy(den[:], cp[HD:HD + 1, :])
                                    rec = scr.tile([1, QT], f32, tag="rec", bufs=2)
                                    rscr = scr.tile([1, QT], f32, tag="rscr", bufs=2)
                                    nc.vector.reciprocal_approx_accurate(
                                        rec[:], den[:], rscr[:])
                                    bc = scr.tile([HD, QT], f32, tag="bc", bufs=2)
                                    nc.gpsimd.partition_broadcast(bc[:], rec[:])
                                    nc.vector.tensor_mul(
                                        ctx[qt][po:po + HD, ft, :], cp[0:HD, :], bc[:])
                                # interleave independent PE work into the bubble
                                for fi in range(hp * nf // 8, (hp + 1) * nf // 8):
                                    fillers[fi]()

                    def outchunk(qt, o):
                        ps = pp.tile([P, QT], f32, tag="pp", name="pso")
                        for dk in range(DK):
                            nc.tensor.matmul(
                                ps[:], wo[:, dk, o, :], ctx[qt][:, dk, :],
                                start=(dk == 0), stop=(dk == DK - 1),
                            )
                        xr = ph2.tile([P, QT], f32, tag="xr", bufs=3, name="xr")
                        nc.sync.dma_start(
                            xr[:], xr_r[:, o, ts(blocks[qt], QT)])
                        nc.vector.scalar_tensor_tensor(
                            yts[qt][:, o, :], ps[:], bo_(o), xr[:],
                            AluOpType.add, AluOpType.add,
                        )

                    def ln_finish(qt):
                        y = yts[qt]
                        mu_ps = pp.tile([1, QT], f32, tag="pp", name="mu_ps")
                        for o in range(DK):
                            nc.tensor.matmul(
                                mu_ps[:], ones128[:], y[:, o, :],
                                start=(o == 0), stop=(o == DK - 1))
                        ms_ps = pp.tile([1, QT], f32, tag="pp", name="ms_ps")
                        for o in range(DK):
                            ysq = ph2.tile([P, QT], bf16, tag="ysq", bufs=2, name="ysq")
                            nc.vector.tensor_mul(ysq[:], y[:, o, :], y[:, o, :])
                            nc.tensor.matmul(
                                ms_ps[:], ones128[:], ysq[:],
                                start=(o == 0), stop=(o == DK - 1))
                        mu = ph2.tile([1, QT], f32, tag="mu_sb", name="mu")
                        nc.scalar.mul(mu[:], mu_ps[:], 1.0 / D)
                        ms = ph2.tile([1, QT], f32, tag="ms_sb", name="ms")
                        nc.scalar.mul(ms[:], ms_ps[:], 1.0 / D)
                        tmp = ph2.tile([1, QT], f32, tag="ltmp", bufs=2, name="tmp")
                        nc.vector.tensor_mul(tmp[:], mu[:], mu[:])
                        nc.vector.tensor_sub(ms[:], ms[:], tmp[:])  # var
                        sd = ph2.tile([1, QT], f32, tag="ltmp", bufs=2, name="sd")
                        nc.scalar.activation(sd[:], ms[:], AF.Sqrt, bias=eps_t[:])
                        rstd = ph2.tile([1, QT], f32, tag="rstd", name="rstd")
                        rsc = ph2.tile([1, QT], f32, tag="ltmp", bufs=2, name="rsc")
                        nc.vector.reciprocal_approx_accurate(rstd[:], sd[:], rsc[:])
                        mu_bc = ph2.tile([P, QT], f32, tag="mu_bc", name="mu_bc")
                        nc.gpsimd.partition_broadcast(mu_bc[:], mu[:])
                        rs_bc = ph2.tile([P, QT], f32, tag="rs_bc", name="rs_bc")
                        nc.gpsimd.partition_broadcast(rs_bc[:], rstd[:])
                        for o in range(DK):
                            t1 = ph2.tile([P, QT], f32, tag="t1", bufs=2, name="t1")
                            nc.vector.tensor_sub(t1[:], y[:, o, :], mu_bc[:])
                            nc.vector.tensor_mul(t1[:], t1[:], rs_bc[:])
                            yo = ph2.tile([P, QT], f32, tag="yo", bufs=2, name="yo")
                            nc.vector.tensor_scalar(
                                yo[:], t1[:], gam_(o), bet_(o),
                                AluOpType.mult, AluOpType.add,
                            )
                            nc.sync.dma_start(yt[ts(o, P), ts(qt, QT)], yo[:])

                    # ---- phase 1: x + weights resident, proj + attn(qt0) --
                    with tc.tile_pool(name=f"ph1_{g}", bufs=1) as ph1:
                        x = ph1.tile([P, DK, S], bf16, name="x")
                        wqs = ph1.tile([P, DK, DK, P], bf16, name="wqs")
                        wvs = ph1.tile([P, DK, 2, 512], bf16, name="wvs")
                        nc.sync.dma_start(wqs[:], wq_r)
                        nc.sync.dma_start(wvs[:], wv_r)
                        for t in range(S // QT):
                            nc.sync.dma_start(
                                x[:, :, ts(t, QT)], xkv_r[:, :, ts(t, QT)])

                        qproj(0)

                        # ---- K^T projection -------------------------------
                        with tc.tile_pool(name=f"wk_{g}", bufs=1) as wkp:
                            wks = wkp.tile([P, DK, DK, P], bf16, name="wks")
                            nc.sync.dma_start(wks[:], wk_r)
                            for t in range(S // QT):
                                for f in range(DK):
                                    ps = pp.tile([P, QT], f32, tag="pp", name="psk")
                                    for dk in range(DK):
                                        nc.tensor.matmul(
                                            ps[:], wks[:, dk, f, :],
                                            x[:, dk, ts(t, QT)],
                                            start=(dk == 0), stop=(dk == DK - 1),
                                        )
                                    nc.vector.tensor_scalar_add(
                                        kt[:, f, ts(t, QT)], ps[:], bk_(f))

                        # ---- V head + attn(qt0) with V-tail/qproj fillers -
                        head = npl[0] + 4
                        for t in range(head):
                            bchunk(t)
                        fillers = [
                            (lambda t=t: bchunk(t)) for t in range(head, NKT)
                        ] + [lambda: qproj(1)]
                        attn(0, fillers)

                    # ---- phase 2: attn(qt1) + out-proj + LN ---------------
                    with tc.tile_pool(name=f"ph2_{g}", bufs=1) as ph2:
                        yts = [
                            ph2.tile([P, DK, QT], bf16, tag="y0", name="y0"),
                            ph2.tile([P, DK, QT], bf16, tag="y1", name="y1"),
                        ]
                        attn(1, [(lambda o=o: outchunk(0, o)) for o in range(DK)])
                        ln_finish(0)
                        for o in range(DK):
                            outchunk(1, o)
                        ln_finish(1)

            pid = nc.partition_id()
            with tc.If(pid < 4) as cmp:
                emit_group(0)
            with cmp.Else():
                emit_group(1)
    nc.compile()
    return nc


def _get_nc():
    if "nc" not in _cache:
        _cache["nc"] = _build()
    return _cache["nc"]


def _prep(x, in_proj_w, in_proj_b, out_w, out_b, gamma, beta):
    import ml_dtypes

    bf16 = ml_dtypes.bfloat16
    x = np.asarray(x, np.float32)
    w = np.asarray(in_proj_w, np.float32)
    wq = np.ascontiguousarray(w[0:D].T.astype(bf16))
    wk = np.ascontiguousarray(w[D:2 * D].T.astype(bf16))
    wv = np.ascontiguousarray(w[2 * D:3 * D].T.astype(bf16))
    wo = np.asarray(out_w, np.float32)
    wot = np.ascontiguousarray(wo.T.astype(bf16))
    bqkv = np.asarray(in_proj_b, np.float32)
    # fold the V bias through the out projection (softmax weights sum to 1)
    bo = (np.asarray(out_b, np.float32)
          + wo @ bqkv[2 * D:3 * D]).astype(np.float32)
    gam = np.asarray(gamma, np.float32)
    bet = np.asarray(beta, np.float32)
    ku = np.arange(P)[:, None] <= (np.arange(896)[None, :] - 384)
    maskd = ku.astype(bf16)
    qcols = {
        0: np.r_[0:QT, 3 * QT:4 * QT],
        1: np.r_[QT:3 * QT],
    }
    in_maps = []
    for c in range(8):
        b = c % 4
        xt = np.ascontiguousarray(x[b].T)
        in_maps.append({
            "xkv": np.ascontiguousarray(xt.astype(bf16)),
            "xrd": xt,
            "wqd": wq, "wkd": wk, "wvd": wv, "wod": wot,
            "maskd": maskd,
            "bqd": bqkv[0:D], "bkd": bqkv[D:2 * D],
            "bod": bo, "gamd": gam, "betd": bet,
        })
    return in_maps, qcols


def _run(in_maps, trace=False, **kw):
    from concourse.bass_utils import run_bass_kernel_spmd

    return run_bass_kernel_spmd(_get_nc(), in_maps, list(range(8)), trace=trace, **kw)


def kernel(x, in_proj_w, in_proj_b, out_w, out_b, gamma, beta):
    in_maps, qcols = _prep(x, in_proj_w, in_proj_b, out_w, out_b, gamma, beta)
    res = _run(in_maps)
    out = np.empty((B, S, D), np.float32)
    for c in range(8):
        out[c % 4, qcols[c // 4]] = res.results[c]["yt"].T
    return out


# revision 6
# speedup vs baseline: 1.8875x; 1.0531x over previous
"""Causal self-attention + residual + LayerNorm fused Trainium2 kernel (v4).

Problem: B=4, S=2048, D=1024, H=16 heads (hd=64), fp32 in/out.
    qkv = x @ in_proj_w.T + in_proj_b ; causal MHA ; out proj ; y = LN(x + attn_out)

Sharding (zero cross-core communication, 8 NeuronCores):
    core c -> batch b = c % 4, query-group g = c // 4.
    Causal zig-zag balance: g=0 owns query blocks [0:512) and [1536:2048),
    g=1 owns [512:1536). Every core computes full K/V for its batch,
    attention only for its own queries, then out-proj + residual +
    LayerNorm for its queries. Outputs are disjoint row sets.

v4 (v1 829us -> v2 462us -> v4): all-bf16 matmul datapath (fp8 measured
over the 2e-2 max-norm gate), plus:
  * Attention starts early: K^T is computed f-block-contiguous (one
    weight-slice DMA per block, all 16 k-tiles), and only f-block 0
    plus the V tiles qt0 needs are computed up front. The remaining
    K^T blocks, V tiles and the qt1 Q-projection are emitted as
    per-head-pair fillers inside attention(qt0) -- f-block hp+1 lands
    in head-pair hp's slot, just in time.
  * out-projection(qt0) chunks fill attention(qt1); out-proj(qt1) is
    emitted before both LayerNorms so its matmuls run during the LN
    DVE chains.
  * Softmax normalize copies the ctx PSUM accumulator to SBUF in one
    op, releasing the PSUM bank ~2us earlier per head pair.
  * Causal mask applied to both heads in one DVE op via a duplicated
    [128, 2, 896] mask tile.
  * exp merged: one ACTIVATE per (head-pair, k-tile) over [128, 2, 512]
    PSUM; in-proj V bias folded into the out-proj bias on the host.
"""
import sys

if "/opt/trn_rl_repo" not in sys.path:
    sys.path.insert(0, "/opt/trn_rl_repo")

import numpy as np

B, S, D, H, HD = 4, 2048, 1024, 16, 64
P = 128
QT = 512                      # queries per q-tile (matmul free dim)
NQ = 1024                     # queries per core
NKT = S // P                  # 16 k-tiles per batch
DK = D // P                   # 8 contraction tiles over D
NPLAIN = {0: (0, 12), 1: (4, 8)}   # group -> per-q-tile plain (unmasked) k-tiles
QBLOCK = {0: (0, 3), 1: (1, 2)}    # group -> 512-col x-block per q-tile

_cache = {}


def _build():
    import concourse.mybir as mybir
    import concourse.tile as tile
    from concourse import bacc
    from concourse.bass import ts
    from concourse.alu_op_type import AluOpType

    f32 = mybir.dt.float32
    bf16 = mybir.dt.bfloat16
    AF = mybir.ActivationFunctionType

    nc = bacc.Bacc("TRN2", target_bir_lowering=False, debug=False, num_devices=8)

    xkv = nc.dram_tensor("xkv", [D, S], bf16, kind="ExternalInput").ap()
    xrd = nc.dram_tensor("xrd", [D, S], f32, kind="ExternalInput").ap()
    wqd = nc.dram_tensor("wqd", [D, D], bf16, kind="ExternalInput").ap()
    wkd = nc.dram_tensor("wkd", [D, D], bf16, kind="ExternalInput").ap()
    wvd = nc.dram_tensor("wvd", [D, D], bf16, kind="ExternalInput").ap()
    wod = nc.dram_tensor("wod", [D, D], bf16, kind="ExternalInput").ap()
    maskd = nc.dram_tensor("maskd", [P, 896], bf16, kind="ExternalInput").ap()
    bqd = nc.dram_tensor("bqd", [D], f32, kind="ExternalInput").ap()
    bkd = nc.dram_tensor("bkd", [D], f32, kind="ExternalInput").ap()
    bod = nc.dram_tensor("bod", [D], f32, kind="ExternalInput").ap()
    gamd = nc.dram_tensor("gamd", [D], f32, kind="ExternalInput").ap()
    betd = nc.dram_tensor("betd", [D], f32, kind="ExternalInput").ap()
    yt = nc.dram_tensor("yt", [D, NQ], f32, kind="ExternalOutput").ap()

    xkv_r = xkv.rearrange("(dk p) t -> p dk t", p=P)
    xr_r = xrd.rearrange("(ok p) t -> p ok t", p=P)
    wq_r = wqd.rearrange("(dk p) (f c) -> p dk f c", p=P, c=P)
    wk_r = wkd.rearrange("(dk p) (f c) -> p dk f c", p=P, c=P)
    wv_r = wvd.rearrange("(dk p) (g c) -> p dk g c", p=P, c=512)
    wo_r = wod.rearrange("(dk p) (o c) -> p dk o c", p=P, c=P)

    with tile.TileContext(nc) as tc:
        with tc.tile_pool(name="pers", bufs=1) as pers:
            kt = pers.tile([P, DK, S], bf16)           # K^T        32 KB/part
            v = pers.tile([P, NKT, H, HD + 1], bf16)   # V aug      33.25 KB
            msk2 = pers.tile([P, 2, 896], bf16)        # mask x2    3.5 KB
            bia = pers.tile([P, DK, 5], f32)           # bq bk bo gam bet
            ones128 = pers.tile([P, 1], bf16)
            eps_t = pers.tile([1, 1], f32)
            wo = pers.tile([P, DK, DK, P], bf16)       # out_w^T    16 KB
            qtl = [
                pers.tile([P, DK, QT], bf16, tag="qtl0", name="qtl0"),
                pers.tile([P, DK, QT], bf16, tag="qtl1", name="qtl1"),
            ]
            ctx = [
                pers.tile([P, DK, QT], bf16, tag="ctx0", name="ctx0"),
                pers.tile([P, DK, QT], bf16, tag="ctx1", name="ctx1"),
            ]

            nc.vector.memset(eps_t[:], 1e-5)
            nc.vector.memset(ones128[:], 1.0)
            nc.vector.memset(v[:, :, :, HD], 1.0)
            nc.sync.dma_start(msk2[:, 0, :], maskd[:])
            nc.sync.dma_start(msk2[:, 1, :], maskd[:])
            nc.sync.dma_start(wo[:], wo_r)
            for j, src in enumerate((bqd, bkd, bod, gamd, betd)):
                nc.sync.dma_start(bia[:, :, j], src.rearrange("(f p) -> p f", p=P))

            def bq_(f): return bia[:, f, 0:1]
            def bk_(f): return bia[:, f, 1:2]
            def bo_(f): return bia[:, f, 2:3]
            def gam_(f): return bia[:, f, 3:4]
            def bet_(f): return bia[:, f, 4:5]

            def emit_group(g):
                npl = NPLAIN[g]
                blocks = QBLOCK[g]

                with tc.tile_pool(name=f"pp{g}", bufs=2, space="PSUM") as pp:

                    def qchunk(qt, fh):
                        xs = x[:, :, ts(blocks[qt], QT)]
                        for f in range(4 * fh, 4 * fh + 4):
                            wsl = ph1.tile([P, DK, P], bf16, tag="wsl",
                                           bufs=3, name="wsl")
                            nc.sync.dma_start(wsl[:], wq_r[:, :, f, :])
                            ps = pp.tile([P, QT], f32, tag="pp", name="psq")
                            for dk in range(DK):
                                nc.tensor.matmul(
                                    ps[:], wsl[:, dk, :], xs[:, dk, :],
                                    start=(dk == 0), stop=(dk == DK - 1),
                                )
                            nc.vector.tensor_scalar_add(
                                qtl[qt][:, f, :], ps[:], bq_(f))

                    def achunk(f):
                        # K^T feature-block f for ALL 16 k-tiles
                        wsl = ph1.tile([P, DK, P], bf16, tag="wsl",
                                       bufs=3, name="wslk")
                        nc.sync.dma_start(wsl[:], wk_r[:, :, f, :])
                        for t in range(S // QT):
                            ps = pp.tile([P, QT], f32, tag="pp", name="psk")
                            for dk in range(DK):
                                nc.tensor.matmul(
                                    ps[:], wsl[:, dk, :], x[:, dk, ts(t, QT)],
                                    start=(dk == 0), stop=(dk == DK - 1),
                                )
                            nc.vector.tensor_scalar_add(
                                kt[:, f, ts(t, QT)], ps[:], bk_(f))

                    def bchunk(t):
                        for fg in range(2):
                            ps = pp.tile([P, 8, HD], f32, tag="pp", name="psv")
                            for dk in range(DK):
                                nc.tensor.matmul(
                                    ps[:], x[:, dk, ts(t, P)], wvs[:, dk, fg, :],
                                    start=(dk == 0), stop=(dk == DK - 1),
                                )
                            nc.scalar.copy(v[:, t, 8 * fg:8 * fg + 8, 0:HD], ps[:])

                    def attn(qt, fillers_by_hp):
                        nk = npl[qt] + 4
                        qtile = qtl[qt]
                        with (
                            tc.tile_pool(name=f"sep{g}{qt}", bufs=4) as sep,
                            tc.tile_pool(name=f"scr{g}{qt}", bufs=1) as scr,
                            tc.tile_pool(name=f"sps{g}{qt}", bufs=2, space="PSUM") as s_ps,
                            tc.tile_pool(name=f"cps{g}{qt}", bufs=1, space="PSUM") as c_ps,
                        ):
                            for hp in range(H // 2):
                                cp0 = c_ps.tile([HD + 1, QT], f32, tag="c0", name="cp0")
                                cp1 = c_ps.tile([HD + 1, QT], f32, tag="c1", name="cp1")
                                for i in range(nk):
                                    sp = s_ps.tile([P, 2, QT], f32, tag="s", name="sp")
                                    se = sep.tile([P, 2, QT], bf16, tag="se", name="se")
                                    nc.tensor.matmul(
                                        sp[:, 0, :], kt[0:HD, hp, ts(i, P)],
                                        qtile[0:HD, hp, :], start=True, stop=True,
                                    )
                                    nc.tensor.matmul(
                                        sp[:, 1, :], kt[HD:P, hp, ts(i, P)],
                                        qtile[HD:P, hp, :], start=True, stop=True,
                                    )
                                    nc.scalar.activation(
                                        se[:], sp[:], AF.Exp, scale=0.125)
                                    if i >= npl[qt]:
                                        off = 384 - P * (i - npl[qt])
                                        nc.vector.tensor_mul(
                                            se[:], se[:],
                                            msk2[:, :, off:off + QT])
                                    nc.tensor.matmul(
                                        cp0[:], v[:, i, 2 * hp, :], se[:, 0, :],
                                        start=(i == 0), stop=(i == nk - 1),
                                    )
                                    nc.tensor.matmul(
                                        cp1[:], v[:, i, 2 * hp + 1, :], se[:, 1, :],
                                        start=(i == 0), stop=(i == nk - 1),
                                    )
                                for j, cp in ((0, cp0), (1, cp1)):
                                    h = 2 * hp + j
                                    po, ft = HD * (h % 2), h // 2
                                    # one copy frees the PSUM bank for the
                                    # next head pair; normalize from SBUF
                                    cr = scr.tile([HD + 1, QT], bf16, tag="cr",
                                                  bufs=2, name="cr")
                                    nc.vector.tensor_copy(cr[:], cp[:])
                                    den = scr.tile([1, QT], f32, tag="den")
                                    nc.vector.tensor_copy(den[:], cr[HD:HD + 1, :])
                                    rec = scr.tile([1, QT], f32, tag="rec")
                                    rscr = scr.tile([1, QT], f32, tag="rscr")
                                    nc.vector.reciprocal_approx_accurate(
                                        rec[:], den[:], rscr[:])
                                    bc = scr.tile([HD, QT], f32, tag="bc", bufs=2)
                                    nc.gpsimd.partition_broadcast(bc[:], rec[:])
                                    nc.vector.tensor_mul(
                                        ctx[qt][po:po + HD, ft, :], cr[0:HD, :], bc[:])
                                for fill in fillers_by_hp[hp]:
                                    fill()

                    def outchunk(qt, o):
                        ps = pp.tile([P, QT], f32, tag="pp", name="pso")
                        for dk in range(DK):
                            nc.tensor.matmul(
                                ps[:], wo[:, dk, o, :], ctx[qt][:, dk, :],
                                start=(dk == 0), stop=(dk == DK - 1),
                            )
                        xr = ph2.tile([P, QT], f32, tag="xr", bufs=3, name="xr")
                        nc.sync.dma_start(
                            xr[:], xr_r[:, o, ts(blocks[qt], QT)])
                        nc.vector.scalar_tensor_tensor(
                            yts[qt][:, o, :], ps[:], bo_(o), xr[:],
                            AluOpType.add, AluOpType.add,
                        )

                    def ln_finish(qt):
                        y = yts[qt]
                        mu_ps = pp.tile([1, QT], f32, tag="pp", name="mu_ps")
                        for o in range(DK):
                            nc.tensor.matmul(
                                mu_ps[:], ones128[:], y[:, o, :],
                                start=(o == 0), stop=(o == DK - 1))
                        ms_ps = pp.tile([1, QT], f32, tag="pp", name="ms_ps")
                        for o in range(DK):
                            ysq = ph2.tile([P, QT], bf16, tag="ysq", bufs=2, name="ysq")
                            nc.vector.tensor_mul(ysq[:], y[:, o, :], y[:, o, :])
                            nc.tensor.matmul(
                                ms_ps[:], ones128[:], ysq[:],
                                start=(o == 0), stop=(o == DK - 1))
                        mu = ph2.tile([1, QT], f32, tag="mu_sb", name="mu")
                        nc.scalar.mul(mu[:], mu_ps[:], 1.0 / D)
                        ms = ph2.tile([1, QT], f32, tag="ms_sb", name="ms")
                        nc.scalar.mul(ms[:], ms_ps[:], 1.0 / D)
                        tmp = ph2.tile([1, QT], f32, tag="ltmp", bufs=2, name="tmp")
                        nc.vector.tensor_mul(tmp[:], mu[:], mu[:])
                        nc.vector.tensor_sub(ms[:], ms[:], tmp[:])  # var
                        sd = ph2.tile([1, QT], f32, tag="ltmp", bufs=2, name="sd")
                        nc.scalar.activation(sd[:], ms[:], AF.Sqrt, bias=eps_t[:])
                        rstd = ph2.tile([1, QT], f32, tag="rstd", name="rstd")
                        rsc = ph2.tile([1, QT], f32, tag="ltmp", bufs=2, name="rsc")
                        nc.vector.reciprocal_approx_accurate(rstd[:], sd[:], rsc[:])
                        mu_bc = ph2.tile([P, QT], f32, tag="mu_bc", name="mu_bc")
                        nc.gpsimd.partition_broadcast(mu_bc[:], mu[:])
                        rs_bc = ph2.tile([P, QT], f32, tag="rs_bc", name="rs_bc")
                        nc.gpsimd.partition_broadcast(rs_bc[:], rstd[:])
                        for o in range(DK):
                            t1 = ph2.tile([P, QT], f32, tag="t1", bufs=2, name="t1")
                            nc.vector.tensor_sub(t1[:], y[:, o, :], mu_bc[:])
                            nc.vector.tensor_mul(t1[:], t1[:], rs_bc[:])
                            yo = ph2.tile([P, QT], f32, tag="yo", bufs=2, name="yo")
                            nc.vector.tensor_scalar(
                                yo[:], t1[:], gam_(o), bet_(o),
                                AluOpType.mult, AluOpType.add,
                            )
                            nc.sync.dma_start(yt[ts(o, P), ts(qt, QT)], yo[:])

                    # ---- phase 1 ------------------------------------------
                    nk0 = npl[0] + 4
                    with tc.tile_pool(name=f"ph1_{g}", bufs=1) as ph1:
                        x = ph1.tile([P, DK, S], bf16, name="x")
                        wvs = ph1.tile([P, DK, 2, 512], bf16, name="wvs")
                        nc.sync.dma_start(wvs[:], wv_r)
                        for t in range(S // QT):
                            nc.sync.dma_start(
                                x[:, :, ts(t, QT)], xkv_r[:, :, ts(t, QT)])

                        qchunk(0, 0)
                        qchunk(0, 1)
                        achunk(0)
                        for t in range(nk0):
                            bchunk(t)

                        # per-head-pair fillers: K^T block hp+1 arrives just
                        # in time; V tail and qt1 Q-proj spread across slots
                        rest = [lambda t=t: bchunk(t) for t in range(nk0, NKT)]
                        rest += [lambda: qchunk(1, 0), lambda: qchunk(1, 1)]
                        fb = [[] for _ in range(8)]
                        for hp in range(7):
                            fb[hp].append(lambda f=hp + 1: achunk(f))
                        for k, r in enumerate(rest):
                            fb[k * 8 // len(rest)].append(r)
                        attn(0, fb)

                    # ---- phase 2 ------------------------------------------
                    with tc.tile_pool(name=f"ph2_{g}", bufs=1) as ph2:
                        yts = [
                            ph2.tile([P, DK, QT], bf16, tag="y0", name="y0"),
                            ph2.tile([P, DK, QT], bf16, tag="y1", name="y1"),
                        ]
                        fb = [[] for _ in range(8)]
                        for o in range(DK):
                            fb[o].append(lambda o=o: outchunk(0, o))
                        attn(1, fb)
                        for o in range(DK):
                            outchunk(1, o)
                        ln_finish(0)
                        ln_finish(1)

            pid = nc.partition_id()
            with tc.If(pid < 4) as cmp:
                emit_group(0)
            with cmp.Else():
                emit_group(1)
    nc.compile()
    return nc


def _get_nc():
    if "nc" not in _cache:
        _cache["nc"] = _build()
    return _cache["nc"]


def _prep(x, in_proj_w, in_proj_b, out_w, out_b, gamma, beta):
    import ml_dtypes

    bf16 = ml_dtypes.bfloat16
    x = np.asarray(x, np.float32)
    w = np.asarray(in_proj_w, np.float32)
    wq = np.ascontiguousarray(w[0:D].T.astype(bf16))
    wk = np.ascontiguousarray(w[D:2 * D].T.astype(bf16))
    wv = np.ascontiguousarray(w[2 * D:3 * D].T.astype(bf16))
    wo = np.asarray(out_w, np.float32)
    wot = np.ascontiguousarray(wo.T.astype(bf16))
    bqkv = np.asarray(in_proj_b, np.float32)
    # fold the V bias through the out projection (softmax weights sum to 1)
    bo_eff = (np.asarray(out_b, np.float32)
              + wo @ bqkv[2 * D:3 * D]).astype(np.float32)
    gam = np.asarray(gamma, np.float32)
    bet = np.asarray(beta, np.float32)
    ku = np.arange(P)[:, None] <= (np.arange(896)[None, :] - 384)
    maskd = ku.astype(bf16)
    qcols = {
        0: np.r_[0:QT, 3 * QT:4 * QT],
        1: np.r_[QT:3 * QT],
    }
    in_maps = []
    for c in range(8):
        b = c % 4
        xt = np.ascontiguousarray(x[b].T)
        in_maps.append({
            "xkv": np.ascontiguousarray(xt.astype(bf16)),
            "xrd": xt,
            "wqd": wq, "wkd": wk, "wvd": wv, "wod": wot,
            "maskd": maskd,
            "bqd": bqkv[0:D], "bkd": bqkv[D:2 * D],
            "bod": bo_eff, "gamd": gam, "betd": bet,
        })
    return in_maps, qcols


def _run(in_maps, trace=False, **kw):
    from concourse.bass_utils import run_bass_kernel_spmd

    return run_bass_kernel_spmd(_get_nc(), in_maps, list(range(8)), trace=trace, **kw)


def kernel(x, in_proj_w, in_proj_b, out_w, out_b, gamma, beta):
    in_maps, qcols = _prep(x, in_proj_w, in_proj_b, out_w, out_b, gamma, beta)
    res = _run(in_maps)
    out = np.empty((B, S, D), np.float32)
    for c in range(8):
        out[c % 4, qcols[c // 4]] = res.results[c]["yt"].T
    return out


# revision 11
# speedup vs baseline: 1.9351x; 1.0252x over previous
"""Causal self-attention + residual + LayerNorm fused Trainium2 kernel (v4).

Problem: B=4, S=2048, D=1024, H=16 heads (hd=64), fp32 in/out.
    qkv = x @ in_proj_w.T + in_proj_b ; causal MHA ; out proj ; y = LN(x + attn_out)

Sharding (zero cross-core communication, 8 NeuronCores):
    core c -> batch b = c % 4, query-group g = c // 4.
    Causal zig-zag balance: g=0 owns query blocks [0:512) and [1536:2048),
    g=1 owns [512:1536). Every core computes full K/V for its batch,
    attention only for its own queries, then out-proj + residual +
    LayerNorm for its queries. Outputs are disjoint row sets.

v4 (v1 829us -> v2 462us -> v4): all-bf16 matmul datapath (fp8 measured
over the 2e-2 max-norm gate), plus:
  * Attention starts early: K^T is computed f-block-contiguous (one
    weight-slice DMA per block, all 16 k-tiles), and only f-block 0
    plus the V tiles qt0 needs are computed up front. The remaining
    K^T blocks, V tiles and the qt1 Q-projection are emitted as
    per-head-pair fillers inside attention(qt0) -- f-block hp+1 lands
    in head-pair hp's slot, just in time.
  * out-projection(qt0) chunks fill attention(qt1); out-proj(qt1) is
    emitted before both LayerNorms so its matmuls run during the LN
    DVE chains.
  * Softmax normalize copies the ctx PSUM accumulator to SBUF in one
    op, releasing the PSUM bank ~2us earlier per head pair.
  * Causal mask applied to both heads in one DVE op via a duplicated
    [128, 2, 896] mask tile.
  * exp merged: one ACTIVATE per (head-pair, k-tile) over [128, 2, 512]
    PSUM; in-proj V bias folded into the out-proj bias on the host.
"""
import sys

if "/opt/trn_rl_repo" not in sys.path:
    sys.path.insert(0, "/opt/trn_rl_repo")

import numpy as np

B, S, D, H, HD = 4, 2048, 1024, 16, 64
P = 128
QT = 512                      # queries per q-tile (matmul free dim)
NQ = 1024                     # queries per core
NKT = S // P                  # 16 k-tiles per batch
DK = D // P                   # 8 contraction tiles over D
NPLAIN = {0: (0, 12), 1: (4, 8)}   # group -> per-q-tile plain (unmasked) k-tiles
QBLOCK = {0: (0, 3), 1: (1, 2)}    # group -> 512-col x-block per q-tile

_cache = {}


def _build():
    import concourse.mybir as mybir
    import concourse.tile as tile
    from concourse import bacc
    from concourse.bass import ts
    from concourse.alu_op_type import AluOpType

    f32 = mybir.dt.float32
    bf16 = mybir.dt.bfloat16
    AF = mybir.ActivationFunctionType

    nc = bacc.Bacc("TRN2", target_bir_lowering=False, debug=False, num_devices=8)

    xkv = nc.dram_tensor("xkv", [D, S], bf16, kind="ExternalInput").ap()
    xrd = nc.dram_tensor("xrd", [D, S], f32, kind="ExternalInput").ap()
    wqd = nc.dram_tensor("wqd", [D, D], bf16, kind="ExternalInput").ap()
    wkd = nc.dram_tensor("wkd", [D, D], bf16, kind="ExternalInput").ap()
    wvd = nc.dram_tensor("wvd", [D, D], bf16, kind="ExternalInput").ap()
    wod = nc.dram_tensor("wod", [D, D], bf16, kind="ExternalInput").ap()
    maskd = nc.dram_tensor("maskd", [P, 896], bf16, kind="ExternalInput").ap()
    bqd = nc.dram_tensor("bqd", [D], f32, kind="ExternalInput").ap()
    bkd = nc.dram_tensor("bkd", [D], f32, kind="ExternalInput").ap()
    bod = nc.dram_tensor("bod", [D], f32, kind="ExternalInput").ap()
    gamd = nc.dram_tensor("gamd", [D], f32, kind="ExternalInput").ap()
    betd = nc.dram_tensor("betd", [D], f32, kind="ExternalInput").ap()
    yt = nc.dram_tensor("yt", [D, NQ], f32, kind="ExternalOutput").ap()

    xkv_r = xkv.rearrange("(dk p) t -> p dk t", p=P)
    xr_r = xrd.rearrange("(ok p) t -> p ok t", p=P)
    wq_r = wqd.rearrange("(dk p) (f c) -> p dk f c", p=P, c=P)
    wk_r = wkd.rearrange("(dk p) (f c) -> p dk f c", p=P, c=P)
    wv_r = wvd.rearrange("(dk p) (g c) -> p dk g c", p=P, c=512)
    wo_r = wod.rearrange("(dk p) (o c) -> p dk o c", p=P, c=P)

    with tile.TileContext(nc) as tc:
        with tc.tile_pool(name="pers", bufs=1) as pers:
            kt = pers.tile([P, DK, S], bf16)           # K^T        32 KB/part
            v = pers.tile([P, NKT, H, HD + 1], bf16)   # V aug      33.25 KB
            msk2 = pers.tile([P, 2, 896], bf16)        # mask x2    3.5 KB
            bia = pers.tile([P, DK, 5], f32)           # bq bk bo gam bet
            ones128 = pers.tile([P, 1], bf16)
            eps_t = pers.tile([1, 1], f32)
            wo = pers.tile([P, DK, DK, P], bf16)       # out_w^T    16 KB
            qtl = [
                pers.tile([P, DK, QT], bf16, tag="qtl0", name="qtl0"),
                pers.tile([P, DK, QT], bf16, tag="qtl1", name="qtl1"),
            ]
            ctx = [
                pers.tile([P, DK, QT], bf16, tag="ctx0", name="ctx0"),
                pers.tile([P, DK, QT], bf16, tag="ctx1", name="ctx1"),
            ]

            nc.vector.memset(eps_t[:], 1e-5)
            nc.vector.memset(ones128[:], 1.0)
            nc.vector.memset(v[:, :, :, HD], 1.0)

            def bq_(f): return bia[:, f, 0:1]
            def bk_(f): return bia[:, f, 1:2]
            def bo_(f): return bia[:, f, 2:3]
            def gam_(f): return bia[:, f, 3:4]
            def bet_(f): return bia[:, f, 4:5]

            def emit_group(g):
                npl = NPLAIN[g]
                blocks = QBLOCK[g]

                with tc.tile_pool(name=f"pp{g}", bufs=2, space="PSUM") as pp:

                    def qchunk(qt, fh):
                        xs = x[:, :, ts(blocks[qt], QT)]
                        for f in range(4 * fh, 4 * fh + 4):
                            wsl = ph1.tile([P, DK, P], bf16, tag="wsl",
                                           bufs=3, name="wsl")
                            nc.sync.dma_start(wsl[:], wq_r[:, :, f, :])
                            ps = pp.tile([P, QT], f32, tag="pp", name="psq")
                            for dk in range(DK):
                                nc.tensor.matmul(
                                    ps[:], wsl[:, dk, :], xs[:, dk, :],
                                    start=(dk == 0), stop=(dk == DK - 1),
                                )
                            nc.vector.tensor_scalar_add(
                                qtl[qt][:, f, :], ps[:], bq_(f))

                    def achunk(f):
                        # K^T feature-block f for ALL 16 k-tiles
                        wsl = ph1.tile([P, DK, P], bf16, tag="wsl",
                                       bufs=3, name="wslk")
                        nc.sync.dma_start(wsl[:], wk_r[:, :, f, :])
                        for t in range(S // QT):
                            ps = pp.tile([P, QT], f32, tag="pp", name="psk")
                            for dk in range(DK):
                                nc.tensor.matmul(
                                    ps[:], wsl[:, dk, :], x[:, dk, ts(t, QT)],
                                    start=(dk == 0), stop=(dk == DK - 1),
                                )
                            nc.vector.tensor_scalar_add(
                                kt[:, f, ts(t, QT)], ps[:], bk_(f))

                    def bchunk(t):
                        for fg in range(2):
                            ps = pp.tile([P, 8, HD], f32, tag="pp", name="psv")
                            for dk in range(DK):
                                nc.tensor.matmul(
                                    ps[:], x[:, dk, ts(t, P)], wvs[:, dk, fg, :],
                                    start=(dk == 0), stop=(dk == DK - 1),
                                )
                            nc.scalar.copy(v[:, t, 8 * fg:8 * fg + 8, 0:HD], ps[:])

                    def attn(qt, fillers_by_hp):
                        nk = npl[qt] + 4
                        qtile = qtl[qt]
                        with (
                            tc.tile_pool(name=f"sep{g}{qt}", bufs=4) as sep,
                            tc.tile_pool(name=f"scr{g}{qt}", bufs=1) as scr,
                            tc.tile_pool(name=f"sps{g}{qt}", bufs=2, space="PSUM") as s_ps,
                            tc.tile_pool(name=f"cps{g}{qt}", bufs=1, space="PSUM") as c_ps,
                        ):
                            for hp in range(H // 2):
                                cp0 = c_ps.tile([HD + 1, QT], f32, tag="c0", name="cp0")
                                cp1 = c_ps.tile([HD + 1, QT], f32, tag="c1", name="cp1")
                                for i in range(nk):
                                    sp = s_ps.tile([P, 2, QT], f32, tag="s", name="sp")
                                    se = sep.tile([P, 2, QT], bf16, tag="se", name="se")
                                    nc.tensor.matmul(
                                        sp[:, 0, :], kt[0:HD, hp, ts(i, P)],
                                        qtile[0:HD, hp, :], start=True, stop=True,
                                    )
                                    nc.tensor.matmul(
                                        sp[:, 1, :], kt[HD:P, hp, ts(i, P)],
                                        qtile[HD:P, hp, :], start=True, stop=True,
                                    )
                                    nc.scalar.activation(
                                        se[:], sp[:], AF.Exp, scale=0.125)
                                    if i >= npl[qt]:
                                        off = 384 - P * (i - npl[qt])
                                        nc.vector.tensor_mul(
                                            se[:], se[:],
                                            msk2[:, :, off:off + QT])
                                    nc.tensor.matmul(
                                        cp0[:], v[:, i, 2 * hp, :], se[:, 0, :],
                                        start=(i == 0), stop=(i == nk - 1),
                                    )
                                    nc.tensor.matmul(
                                        cp1[:], v[:, i, 2 * hp + 1, :], se[:, 1, :],
                                        start=(i == 0), stop=(i == nk - 1),
                                    )
                                for j, cp in ((0, cp0), (1, cp1)):
                                    h = 2 * hp + j
                                    po, ft = HD * (h % 2), h // 2
                                    # one copy frees the PSUM bank for the
                                    # next head pair; normalize from SBUF
                                    cr = scr.tile([HD + 1, QT], bf16, tag="cr",
                                                  bufs=2, name="cr")
                                    nc.vector.tensor_copy(cr[:], cp[:])
                                    den = scr.tile([1, QT], f32, tag="den")
                                    nc.vector.tensor_copy(den[:], cr[HD:HD + 1, :])
                                    rec = scr.tile([1, QT], f32, tag="rec")
                                    rscr = scr.tile([1, QT], f32, tag="rscr")
                                    nc.vector.reciprocal_approx_accurate(
                                        rec[:], den[:], rscr[:])
                                    bc = scr.tile([HD, QT], f32, tag="bc", bufs=2)
                                    nc.gpsimd.partition_broadcast(bc[:], rec[:])
                                    nc.vector.tensor_mul(
                                        ctx[qt][po:po + HD, ft, :], cr[0:HD, :], bc[:])
                                for fill in fillers_by_hp[hp]:
                                    fill()

                    def outchunk(qt, o):
                        ps = pp.tile([P, QT], f32, tag="pp", name="pso")
                        for dk in range(DK):
                            nc.tensor.matmul(
                                ps[:], wo[:, dk, o, :], ctx[qt][:, dk, :],
                                start=(dk == 0), stop=(dk == DK - 1),
                            )
                        xr = ph2.tile([P, QT], f32, tag="xr", bufs=3, name="xr")
                        nc.sync.dma_start(
                            xr[:], xr_r[:, o, ts(blocks[qt], QT)])
                        nc.vector.scalar_tensor_tensor(
                            yts[qt][:, o, :], ps[:], bo_(o), xr[:],
                            AluOpType.add, AluOpType.add,
                        )

                    def stats_chunk(qt, o, st):
                        y = yts[qt]
                        mu_ps, ms_ps = st
                        nc.tensor.matmul(
                            mu_ps[:], ones128[:], y[:, o, :],
                            start=(o == 0), stop=(o == DK - 1))
                        ysq = ph2.tile([P, QT], bf16, tag="ysq", bufs=2, name="ysq")
                        nc.vector.tensor_mul(ysq[:], y[:, o, :], y[:, o, :])
                        nc.tensor.matmul(
                            ms_ps[:], ones128[:], ysq[:],
                            start=(o == 0), stop=(o == DK - 1))

                    def ln_chain(qt, st):
                        mu_ps, ms_ps = st
                        mu = ph2.tile([1, QT], f32, tag=f"mu{qt}", name="mu")
                        nc.scalar.mul(mu[:], mu_ps[:], 1.0 / D)
                        ms = ph2.tile([1, QT], f32, tag=f"ms{qt}", name="ms")
                        nc.scalar.mul(ms[:], ms_ps[:], 1.0 / D)
                        tmp = ph2.tile([1, QT], f32, tag="ltmp", bufs=2, name="tmp")
                        nc.vector.tensor_mul(tmp[:], mu[:], mu[:])
                        nc.vector.tensor_sub(ms[:], ms[:], tmp[:])  # var
                        sd = ph2.tile([1, QT], f32, tag="ltmp", bufs=2, name="sd")
                        nc.scalar.activation(sd[:], ms[:], AF.Sqrt, bias=eps_t[:])
                        rstd = ph2.tile([1, QT], f32, tag=f"rstd{qt}", name="rstd")
                        rsc = ph2.tile([1, QT], f32, tag="ltmp", bufs=2, name="rsc")
                        nc.vector.reciprocal_approx_accurate(rstd[:], sd[:], rsc[:])
                        mu_bc = ph2.tile([P, QT], f32, tag=f"mu_bc{qt}", name="mu_bc")
                        nc.gpsimd.partition_broadcast(mu_bc[:], mu[:])
                        rs_bc = ph2.tile([P, QT], f32, tag=f"rs_bc{qt}", name="rs_bc")
                        nc.gpsimd.partition_broadcast(rs_bc[:], rstd[:])
                        return mu_bc, rs_bc

                    def ln_final(qt, o, mu_bc, rs_bc):
                        t1 = ph2.tile([P, QT], bf16, tag="t1", bufs=2, name="t1")
                        nc.vector.tensor_sub(t1[:], yts[qt][:, o, :], mu_bc[:])
                        nc.vector.tensor_mul(t1[:], t1[:], rs_bc[:])
                        yo = ph2.tile([P, QT], f32, tag="yo", bufs=2, name="yo")
                        nc.vector.tensor_scalar(
                            yo[:], t1[:], gam_(o), bet_(o),
                            AluOpType.mult, AluOpType.add,
                        )
                        nc.sync.dma_start(yt[ts(o, P), ts(qt, QT)], yo[:])

                    # ---- phase 1 ------------------------------------------
                    nk0 = npl[0] + 4
                    with tc.tile_pool(name=f"ph1_{g}", bufs=1) as ph1:
                        x = ph1.tile([P, DK, S], bf16, name="x")
                        wvs = ph1.tile([P, DK, 2, 512], bf16, name="wvs")
                        # x block for qproj(0) first, then the rest; small
                        # tables next; wo (first needed ~200us in) last
                        nc.sync.dma_start(
                            x[:, :, ts(blocks[0], QT)],
                            xkv_r[:, :, ts(blocks[0], QT)])
                        for j, src in enumerate((bqd, bkd, bod, gamd, betd)):
                            nc.sync.dma_start(
                                bia[:, :, j], src.rearrange("(f p) -> p f", p=P))
                        for t in range(S // QT):
                            if t != blocks[0]:
                                nc.sync.dma_start(
                                    x[:, :, ts(t, QT)], xkv_r[:, :, ts(t, QT)])
                        nc.sync.dma_start(msk2[:, 0, :], maskd[:])
                        nc.sync.dma_start(msk2[:, 1, :], maskd[:])
                        nc.sync.dma_start(wvs[:], wv_r)
                        nc.sync.dma_start(wo[:], wo_r)

                        qchunk(0, 0)
                        qchunk(0, 1)
                        achunk(0)
                        for t in range(nk0):
                            bchunk(t)

                        # per-head-pair fillers: K^T block hp+1 arrives just
                        # in time; V tail and qt1 Q-proj spread across slots
                        rest = [lambda t=t: bchunk(t) for t in range(nk0, NKT)]
                        rest += [lambda: qchunk(1, 0), lambda: qchunk(1, 1)]
                        fb = [[] for _ in range(8)]
                        for hp in range(7):
                            fb[hp].append(lambda f=hp + 1: achunk(f))
                        for k, r in enumerate(rest):
                            fb[k * 8 // len(rest)].append(r)
                        attn(0, fb)

                    # ---- phase 2 ------------------------------------------
                    with tc.tile_pool(name=f"ph2_{g}", bufs=1) as ph2:
                        yts = [
                            ph2.tile([P, DK, QT], bf16, tag="y0", name="y0"),
                            ph2.tile([P, DK, QT], bf16, tag="y1", name="y1"),
                        ]
                        fb = [[] for _ in range(8)]
                        for o in range(DK):
                            fb[o].append(lambda o=o: outchunk(0, o))
                        attn(1, fb)
                        # tail: qt0 stats ride along out-proj(qt1); ln0
                        # finals overlap qt1 stats; only ln1 is serial
                        with tc.tile_pool(name=f"st_{g}", bufs=1,
                                          space="PSUM") as stp:
                            st = [
                                (stp.tile([1, QT], f32, tag="mu0", name="mu0p"),
                                 stp.tile([1, QT], f32, tag="ms0", name="ms0p")),
                                (stp.tile([1, QT], f32, tag="mu1", name="mu1p"),
                                 stp.tile([1, QT], f32, tag="ms1", name="ms1p")),
                            ]
                            for o in range(DK):
                                outchunk(1, o)
                                stats_chunk(0, o, st[0])
                            bc0 = ln_chain(0, st[0])
                            for o in range(DK):
                                ln_final(0, o, *bc0)
                                stats_chunk(1, o, st[1])
                            bc1 = ln_chain(1, st[1])
                            for o in range(DK):
                                ln_final(1, o, *bc1)

            pid = nc.partition_id()
            with tc.If(pid < 4) as cmp:
                emit_group(0)
            with cmp.Else():
                emit_group(1)
    nc.compile()
    return nc


def _get_nc():
    if "nc" not in _cache:
        _cache["nc"] = _build()
    return _cache["nc"]


def _prep(x, in_proj_w, in_proj_b, out_w, out_b, gamma, beta):
    import ml_dtypes

    bf16 = ml_dtypes.bfloat16
    x = np.asarray(x, np.float32)
    w = np.asarray(in_proj_w, np.float32)
    wq = np.ascontiguousarray(w[0:D].T.astype(bf16))
    wk = np.ascontiguousarray(w[D:2 * D].T.astype(bf16))
    wv = np.ascontiguousarray(w[2 * D:3 * D].T.astype(bf16))
    wo = np.asarray(out_w, np.float32)
    wot = np.ascontiguousarray(wo.T.astype(bf16))
    bqkv = np.asarray(in_proj_b, np.float32)
    # fold the V bias through the out projection (softmax weights sum to 1)
    bo_eff = (np.asarray(out_b, np.float32)
              + wo @ bqkv[2 * D:3 * D]).astype(np.float32)
    gam = np.asarray(gamma, np.float32)
    bet = np.asarray(beta, np.float32)
    ku = np.arange(P)[:, None] <= (np.arange(896)[None, :] - 384)
    maskd = ku.astype(bf16)
    qcols = {
        0: np.r_[0:QT, 3 * QT:4 * QT],
        1: np.r_[QT:3 * QT],
    }
    in_maps = []
    for c in range(8):
        b = c % 4
        xt = np.ascontiguousarray(x[b].T)
        in_maps.append({
            "xkv": np.ascontiguousarray(xt.astype(bf16)),
            "xrd": xt,
            "wqd": wq, "wkd": wk, "wvd": wv, "wod": wot,
            "maskd": maskd,
            "bqd": bqkv[0:D], "bkd": bqkv[D:2 * D],
            "bod": bo_eff, "gamd": gam, "betd": bet,
        })
    return in_maps, qcols


def _run(in_maps, trace=False, **kw):
    from concourse.bass_utils import run_bass_kernel_spmd

    return run_bass_kernel_spmd(_get_nc(), in_maps, list(range(8)), trace=trace, **kw)


def kernel(x, in_proj_w, in_proj_b, out_w, out_b, gamma, beta):
    in_maps, qcols = _prep(x, in_proj_w, in_proj_b, out_w, out_b, gamma, beta)
    res = _run(in_maps)
    out = np.empty((B, S, D), np.float32)
    for c in range(8):
        out[c % 4, qcols[c // 4]] = res.results[c]["yt"].T
    return out


# revision 16
# speedup vs baseline: 1.9880x; 1.0273x over previous
"""Causal self-attention + residual + LayerNorm fused Trainium2 kernel (v4).

Problem: B=4, S=2048, D=1024, H=16 heads (hd=64), fp32 in/out.
    qkv = x @ in_proj_w.T + in_proj_b ; causal MHA ; out proj ; y = LN(x + attn_out)

Sharding (zero cross-core communication, 8 NeuronCores):
    core c -> batch b = c % 4, query-group g = c // 4.
    Causal zig-zag balance: g=0 owns query blocks [0:512) and [1536:2048),
    g=1 owns [512:1536). Every core computes full K/V for its batch,
    attention only for its own queries, then out-proj + residual +
    LayerNorm for its queries. Outputs are disjoint row sets.

v4 (v1 829us -> v2 462us -> v4): all-bf16 matmul datapath (fp8 measured
over the 2e-2 max-norm gate), plus:
  * Attention starts early: K^T is computed f-block-contiguous (one
    weight-slice DMA per block, all 16 k-tiles), and only f-block 0
    plus the V tiles qt0 needs are computed up front. The remaining
    K^T blocks, V tiles and the qt1 Q-projection are emitted as
    per-head-pair fillers inside attention(qt0) -- f-block hp+1 lands
    in head-pair hp's slot, just in time.
  * out-projection(qt0) chunks fill attention(qt1); out-proj(qt1) is
    emitted before both LayerNorms so its matmuls run during the LN
    DVE chains.
  * Softmax normalize copies the ctx PSUM accumulator to SBUF in one
    op, releasing the PSUM bank ~2us earlier per head pair.
  * Causal mask applied to both heads in one DVE op via a duplicated
    [128, 2, 896] mask tile.
  * exp merged: one ACTIVATE per (head-pair, k-tile) over [128, 2, 512]
    PSUM; in-proj V bias folded into the out-proj bias on the host.
"""
import sys

if "/opt/trn_rl_repo" not in sys.path:
    sys.path.insert(0, "/opt/trn_rl_repo")

import numpy as np

B, S, D, H, HD = 4, 2048, 1024, 16, 64
P = 128
QT = 512                      # queries per q-tile (matmul free dim)
NQ = 1024                     # queries per core
NKT = S // P                  # 16 k-tiles per batch
DK = D // P                   # 8 contraction tiles over D
NPLAIN = {0: (0, 12), 1: (4, 8)}   # group -> per-q-tile plain (unmasked) k-tiles
QBLOCK = {0: (0, 3), 1: (1, 2)}    # group -> 512-col x-block per q-tile

_cache = {}


def _build():
    import concourse.mybir as mybir
    import concourse.tile as tile
    from concourse import bacc
    from concourse.bass import ts
    from concourse.alu_op_type import AluOpType

    f32 = mybir.dt.float32
    bf16 = mybir.dt.bfloat16
    AF = mybir.ActivationFunctionType

    nc = bacc.Bacc("TRN2", target_bir_lowering=False, debug=False, num_devices=8)

    xkv = nc.dram_tensor("xkv", [D, S], bf16, kind="ExternalInput").ap()
    xrd = nc.dram_tensor("xrd", [D, S], f32, kind="ExternalInput").ap()
    wqd = nc.dram_tensor("wqd", [D, D], bf16, kind="ExternalInput").ap()
    wkd = nc.dram_tensor("wkd", [D, D], bf16, kind="ExternalInput").ap()
    wvd = nc.dram_tensor("wvd", [D, D], bf16, kind="ExternalInput").ap()
    wod = nc.dram_tensor("wod", [D, D], bf16, kind="ExternalInput").ap()
    maskd = nc.dram_tensor("maskd", [P, 896], bf16, kind="ExternalInput").ap()
    bqd = nc.dram_tensor("bqd", [D], f32, kind="ExternalInput").ap()
    bkd = nc.dram_tensor("bkd", [D], f32, kind="ExternalInput").ap()
    bod = nc.dram_tensor("bod", [D], f32, kind="ExternalInput").ap()
    gamd = nc.dram_tensor("gamd", [D], f32, kind="ExternalInput").ap()
    betd = nc.dram_tensor("betd", [D], f32, kind="ExternalInput").ap()
    yt = nc.dram_tensor("yt", [D, NQ], f32, kind="ExternalOutput").ap()

    xkv_r = xkv.rearrange("(dk p) t -> p dk t", p=P)
    xr_r = xrd.rearrange("(ok p) t -> p ok t", p=P)
    wq_r = wqd.rearrange("(dk p) (f c) -> p dk f c", p=P, c=P)
    wk_r = wkd.rearrange("(dk p) (f c) -> p dk f c", p=P, c=P)
    wv_r = wvd.rearrange("(dk p) (g c) -> p dk g c", p=P, c=512)
    wo_r = wod.rearrange("(dk p) (o c) -> p dk o c", p=P, c=P)

    with tile.TileContext(nc) as tc:
        with tc.tile_pool(name="pers", bufs=1) as pers:
            kt = pers.tile([P, DK, S], bf16)           # K^T        32 KB/part
            v = pers.tile([P, NKT, H, HD + 1], bf16)   # V aug      33.25 KB
            msk2 = pers.tile([P, 2, 896], bf16)        # mask x2    3.5 KB
            bia = pers.tile([P, DK, 5], f32)           # bq bk bo gam bet
            ones128 = pers.tile([P, 1], bf16)
            eps_t = pers.tile([1, 1], f32)
            wo = pers.tile([P, DK, DK, P], bf16)       # out_w^T    16 KB
            qtl = [
                pers.tile([P, DK, QT], bf16, tag="qtl0", name="qtl0"),
                pers.tile([P, DK, QT], bf16, tag="qtl1", name="qtl1"),
            ]
            # per-f-block ctx tiles so out-proj matmuls only depend on the
            # head pairs they actually read (tile-granular deps otherwise
            # serialize out-proj behind the last head's normalize)
            ctx = [
                [pers.tile([P, QT], bf16, tag=f"ctx{qt}_{ft}",
                           name=f"ctx{qt}_{ft}") for ft in range(DK)]
                for qt in range(2)
            ]

            nc.vector.memset(eps_t[:], 1e-5)
            nc.vector.memset(ones128[:], 1.0)
            nc.vector.memset(v[:, :, :, HD], 1.0)

            def bq_(f): return bia[:, f, 0:1]
            def bk_(f): return bia[:, f, 1:2]
            def bo_(f): return bia[:, f, 2:3]
            def gam_(f): return bia[:, f, 3:4]
            def bet_(f): return bia[:, f, 4:5]

            def emit_group(g):
                npl = NPLAIN[g]
                blocks = QBLOCK[g]

                with tc.tile_pool(name=f"pp{g}", bufs=2, space="PSUM") as pp:

                    def qchunk(qt, fh):
                        xs = x[:, :, ts(blocks[qt], QT)]
                        for f in range(4 * fh, 4 * fh + 4):
                            wsl = ph1.tile([P, DK, P], bf16, tag="wsl",
                                           bufs=3, name="wsl")
                            nc.sync.dma_start(wsl[:], wq_r[:, :, f, :])
                            ps = pp.tile([P, QT], f32, tag="pp", name="psq")
                            for dk in range(DK):
                                nc.tensor.matmul(
                                    ps[:], wsl[:, dk, :], xs[:, dk, :],
                                    start=(dk == 0), stop=(dk == DK - 1),
                                )
                            nc.vector.tensor_scalar_add(
                                qtl[qt][:, f, :], ps[:], bq_(f))

                    def achunk(f):
                        # K^T feature-block f for ALL 16 k-tiles
                        wsl = ph1.tile([P, DK, P], bf16, tag="wsl",
                                       bufs=3, name="wslk")
                        nc.sync.dma_start(wsl[:], wk_r[:, :, f, :])
                        for t in range(S // QT):
                            ps = pp.tile([P, QT], f32, tag="pp", name="psk")
                            for dk in range(DK):
                                nc.tensor.matmul(
                                    ps[:], wsl[:, dk, :], x[:, dk, ts(t, QT)],
                                    start=(dk == 0), stop=(dk == DK - 1),
                                )
                            nc.vector.tensor_scalar_add(
                                kt[:, f, ts(t, QT)], ps[:], bk_(f))

                    def bchunk(t):
                        for fg in range(2):
                            ps = pp.tile([P, 8, HD], f32, tag="pp", name="psv")
                            for dk in range(DK):
                                nc.tensor.matmul(
                                    ps[:], x[:, dk, ts(t, P)], wvs[:, dk, fg, :],
                                    start=(dk == 0), stop=(dk == DK - 1),
                                )
                            nc.scalar.copy(v[:, t, 8 * fg:8 * fg + 8, 0:HD], ps[:])

                    def attn(qt, fillers_by_hp):
                        nk = npl[qt] + 4
                        qtile = qtl[qt]
                        with (
                            tc.tile_pool(name=f"sep{g}{qt}", bufs=4) as sep,
                            tc.tile_pool(name=f"scr{g}{qt}", bufs=1) as scr,
                            tc.tile_pool(name=f"sps{g}{qt}", bufs=2, space="PSUM") as s_ps,
                            tc.tile_pool(name=f"cps{g}{qt}", bufs=1, space="PSUM") as c_ps,
                        ):
                            for hp in range(H // 2):
                                cp0 = c_ps.tile([HD + 1, QT], f32, tag="c0", name="cp0")
                                cp1 = c_ps.tile([HD + 1, QT], f32, tag="c1", name="cp1")
                                for i in range(nk):
                                    # queries below the diagonal band see
                                    # every key of this k-tile masked; skip
                                    # those columns in S, exp, mask and ctx
                                    lo = max(0, P * (i - npl[qt]))
                                    sp = s_ps.tile([P, 2, QT], f32, tag="s", name="sp")
                                    se = sep.tile([P, 2, QT], bf16, tag="se", name="se")
                                    nc.tensor.matmul(
                                        sp[:, 0, lo:QT], kt[0:HD, hp, ts(i, P)],
                                        qtile[0:HD, hp, lo:QT],
                                        start=True, stop=True,
                                    )
                                    nc.tensor.matmul(
                                        sp[:, 1, lo:QT], kt[HD:P, hp, ts(i, P)],
                                        qtile[HD:P, hp, lo:QT],
                                        start=True, stop=True,
                                    )
                                    nc.scalar.activation(
                                        se[:, :, lo:QT], sp[:, :, lo:QT],
                                        AF.Exp, scale=0.125)
                                    if i >= npl[qt]:
                                        nc.vector.tensor_mul(
                                            se[:, :, lo:QT], se[:, :, lo:QT],
                                            msk2[:, :, 384:384 + QT - lo])
                                    nc.tensor.matmul(
                                        cp0[:, lo:QT], v[:, i, 2 * hp, :],
                                        se[:, 0, lo:QT],
                                        start=(i == 0), stop=(i == nk - 1),
                                    )
                                    nc.tensor.matmul(
                                        cp1[:, lo:QT], v[:, i, 2 * hp + 1, :],
                                        se[:, 1, lo:QT],
                                        start=(i == 0), stop=(i == nk - 1),
                                    )
                                for j, cp in ((0, cp0), (1, cp1)):
                                    h = 2 * hp + j
                                    po, ft = HD * (h % 2), h // 2
                                    # one copy frees the PSUM bank for the
                                    # next head pair; normalize from SBUF
                                    cr = scr.tile([HD + 1, QT], bf16, tag="cr",
                                                  bufs=2, name="cr")
                                    nc.vector.tensor_copy(cr[:], cp[:])
                                    den = scr.tile([1, QT], f32, tag="den")
                                    nc.vector.tensor_copy(den[:], cr[HD:HD + 1, :])
                                    rec = scr.tile([1, QT], f32, tag="rec")
                                    rscr = scr.tile([1, QT], f32, tag="rscr")
                                    nc.vector.reciprocal_approx_accurate(
                                        rec[:], den[:], rscr[:])
                                    bc = scr.tile([HD, QT], f32, tag="bc", bufs=2)
                                    nc.gpsimd.partition_broadcast(bc[:], rec[:])
                                    nc.vector.tensor_mul(
                                        ctx[qt][ft][po:po + HD, :], cr[0:HD, :], bc[:])
                                for fill in fillers_by_hp[hp]:
                                    fill()

                    def outchunk(qt, o):
                        ps = pp.tile([P, QT], f32, tag="pp", name="pso")
                        for dk in range(DK):
                            nc.tensor.matmul(
                                ps[:], wo[:, dk, o, :], ctx[qt][dk][:, :],
                                start=(dk == 0), stop=(dk == DK - 1),
                            )
                        xr = ph2.tile([P, QT], f32, tag="xr", bufs=3, name="xr")
                        nc.sync.dma_start(
                            xr[:], xr_r[:, o, ts(blocks[qt], QT)])
                        nc.vector.scalar_tensor_tensor(
                            yts[qt][:, o, :], ps[:], bo_(o), xr[:],
                            AluOpType.add, AluOpType.add,
                        )

                    def stats_chunk(qt, o, st):
                        y = yts[qt]
                        mu_ps, ms_ps = st
                        nc.tensor.matmul(
                            mu_ps[:], ones128[:], y[:, o, :],
                            start=(o == 0), stop=(o == DK - 1))
                        ysq = ph2.tile([P, QT], bf16, tag="ysq", bufs=2, name="ysq")
                        nc.vector.tensor_mul(ysq[:], y[:, o, :], y[:, o, :])
                        nc.tensor.matmul(
                            ms_ps[:], ones128[:], ysq[:],
                            start=(o == 0), stop=(o == DK - 1))

                    def ln_chain(qt, st):
                        mu_ps, ms_ps = st
                        mu = ph2.tile([1, QT], f32, tag=f"mu{qt}", name="mu")
                        nc.scalar.mul(mu[:], mu_ps[:], 1.0 / D)
                        ms = ph2.tile([1, QT], f32, tag=f"ms{qt}", name="ms")
                        nc.scalar.mul(ms[:], ms_ps[:], 1.0 / D)
                        tmp = ph2.tile([1, QT], f32, tag="ltmp", bufs=2, name="tmp")
                        nc.vector.tensor_mul(tmp[:], mu[:], mu[:])
                        nc.vector.tensor_sub(ms[:], ms[:], tmp[:])  # var
                        sd = ph2.tile([1, QT], f32, tag="ltmp", bufs=2, name="sd")
                        nc.scalar.activation(sd[:], ms[:], AF.Sqrt, bias=eps_t[:])
                        rstd = ph2.tile([1, QT], f32, tag=f"rstd{qt}", name="rstd")
                        rsc = ph2.tile([1, QT], f32, tag="ltmp", bufs=2, name="rsc")
                        nc.vector.reciprocal_approx_accurate(rstd[:], sd[:], rsc[:])
                        mu_bc = ph2.tile([P, QT], f32, tag=f"mu_bc{qt}", name="mu_bc")
                        nc.gpsimd.partition_broadcast(mu_bc[:], mu[:])
                        rs_bc = ph2.tile([P, QT], f32, tag=f"rs_bc{qt}", name="rs_bc")
                        nc.gpsimd.partition_broadcast(rs_bc[:], rstd[:])
                        return mu_bc, rs_bc

                    def ln_final(qt, o, mu_bc, rs_bc):
                        t1 = ph2.tile([P, QT], bf16, tag="t1", bufs=2, name="t1")
                        nc.vector.tensor_sub(t1[:], yts[qt][:, o, :], mu_bc[:])
                        nc.vector.tensor_mul(t1[:], t1[:], rs_bc[:])
                        yo = ph2.tile([P, QT], f32, tag="yo", bufs=2, name="yo")
                        nc.vector.tensor_scalar(
                            yo[:], t1[:], gam_(o), bet_(o),
                            AluOpType.mult, AluOpType.add,
                        )
                        nc.sync.dma_start(yt[ts(o, P), ts(qt, QT)], yo[:])

                    # ---- phase 1 ------------------------------------------
                    nk0 = npl[0] + 4
                    with tc.tile_pool(name=f"ph1_{g}", bufs=1) as ph1:
                        x = ph1.tile([P, DK, S], bf16, name="x")
                        wvs = ph1.tile([P, DK, 2, 512], bf16, name="wvs")
                        # x block for qproj(0) first, then the rest; small
                        # tables next; wo (first needed ~200us in) last
                        nc.sync.dma_start(
                            x[:, :, ts(blocks[0], QT)],
                            xkv_r[:, :, ts(blocks[0], QT)])
                        for j, src in enumerate((bqd, bkd, bod, gamd, betd)):
                            nc.sync.dma_start(
                                bia[:, :, j], src.rearrange("(f p) -> p f", p=P))
                        for t in range(S // QT):
                            if t != blocks[0]:
                                nc.sync.dma_start(
                                    x[:, :, ts(t, QT)], xkv_r[:, :, ts(t, QT)])
                        nc.sync.dma_start(msk2[:, 0, :], maskd[:])
                        nc.sync.dma_start(msk2[:, 1, :], maskd[:])
                        nc.sync.dma_start(wvs[:], wv_r)
                        nc.sync.dma_start(wo[:], wo_r)

                        qchunk(0, 0)
                        qchunk(0, 1)
                        achunk(0)
                        for t in range(nk0):
                            bchunk(t)

                        # per-head-pair fillers: K^T block hp+1 arrives just
                        # in time; V tail and qt1 Q-proj spread across slots
                        rest = [lambda t=t: bchunk(t) for t in range(nk0, NKT)]
                        rest += [lambda: qchunk(1, 0), lambda: qchunk(1, 1)]
                        fb = [[] for _ in range(8)]
                        for hp in range(7):
                            fb[hp].append(lambda f=hp + 1: achunk(f))
                        for k, r in enumerate(rest):
                            fb[k * 8 // len(rest)].append(r)
                        attn(0, fb)

                    # ---- phase 2 ------------------------------------------
                    with tc.tile_pool(name=f"ph2_{g}", bufs=1) as ph2:
                        yts = [
                            ph2.tile([P, DK, QT], bf16, tag="y0", name="y0"),
                            ph2.tile([P, DK, QT], bf16, tag="y1", name="y1"),
                        ]
                        # out-proj(qt0) fillers early (they only need ctx0);
                        # the last slot computes qt0's LN statistics in the
                        # pp PSUM ring, filling the final head's bubble
                        st0 = []

                        def stats0():
                            st0.append((
                                pp.tile([1, QT], f32, tag="pp", name="mu0p"),
                                pp.tile([1, QT], f32, tag="pp", name="ms0p")))
                            for o in range(DK):
                                stats_chunk(0, o, st0[0])

                        fb = [[] for _ in range(8)]
                        for o in range(DK):
                            fb[min(o, 6)].append(lambda o=o: outchunk(0, o))
                        fb[7].append(stats0)
                        attn(1, fb)
                        # tail: ln0 chain + finals overlap out-proj(qt1)
                        # and qt1 stats; only the ln1 chain is serial
                        with tc.tile_pool(name=f"st_{g}", bufs=1,
                                          space="PSUM") as stp:
                            st1 = (stp.tile([1, QT], f32, tag="mu1", name="mu1p"),
                                   stp.tile([1, QT], f32, tag="ms1", name="ms1p"))
                            bc0 = ln_chain(0, st0[0])
                            for o in range(DK):
                                outchunk(1, o)
                                ln_final(0, o, *bc0)
                                stats_chunk(1, o, st1)
                            bc1 = ln_chain(1, st1)
                            for o in range(DK):
                                ln_final(1, o, *bc1)

            pid = nc.partition_id()
            with tc.If(pid < 4) as cmp:
                emit_group(0)
            with cmp.Else():
                emit_group(1)
    nc.compile()
    return nc


def _get_nc():
    if "nc" not in _cache:
        _cache["nc"] = _build()
    return _cache["nc"]


def _prep(x, in_proj_w, in_proj_b, out_w, out_b, gamma, beta):
    import ml_dtypes

    bf16 = ml_dtypes.bfloat16
    x = np.asarray(x, np.float32)
    w = np.asarray(in_proj_w, np.float32)
    wq = np.ascontiguousarray(w[0:D].T.astype(bf16))
    wk = np.ascontiguousarray(w[D:2 * D].T.astype(bf16))
    wv = np.ascontiguousarray(w[2 * D:3 * D].T.astype(bf16))
    wo = np.asarray(out_w, np.float32)
    wot = np.ascontiguousarray(wo.T.astype(bf16))
    bqkv = np.asarray(in_proj_b, np.float32)
    # fold the V bias through the out projection (softmax weights sum to 1)
    bo_eff = (np.asarray(out_b, np.float32)
              + wo @ bqkv[2 * D:3 * D]).astype(np.float32)
    gam = np.asarray(gamma, np.float32)
    bet = np.asarray(beta, np.float32)
    ku = np.arange(P)[:, None] <= (np.arange(896)[None, :] - 384)
    maskd = ku.astype(bf16)
    qcols = {
        0: np.r_[0:QT, 3 * QT:4 * QT],
        1: np.r_[QT:3 * QT],
    }
    in_maps = []
    for c in range(8):
        b = c % 4
        xt = np.ascontiguousarray(x[b].T)
        in_maps.append({
            "xkv": np.ascontiguousarray(xt.astype(bf16)),
            "xrd": xt,
            "wqd": wq, "wkd": wk, "wvd": wv, "wod": wot,
            "maskd": maskd,
            "bqd": bqkv[0:D], "bkd": bqkv[D:2 * D],
            "bod": bo_eff, "gamd": gam, "betd": bet,
        })
    return in_maps, qcols


def _run(in_maps, trace=False, **kw):
    from concourse.bass_utils import run_bass_kernel_spmd

    return run_bass_kernel_spmd(_get_nc(), in_maps, list(range(8)), trace=trace, **kw)


def kernel(x, in_proj_w, in_proj_b, out_w, out_b, gamma, beta):
    in_maps, qcols = _prep(x, in_proj_w, in_proj_b, out_w, out_b, gamma, beta)
    res = _run(in_maps)
    out = np.empty((B, S, D), np.float32)
    for c in range(8):
        out[c % 4, qcols[c // 4]] = res.results[c]["yt"].T
    return out


# revision 17
# speedup vs baseline: 1.9897x; 1.0009x over previous
"""Causal self-attention + residual + LayerNorm fused Trainium2 kernel (v4).

Problem: B=4, S=2048, D=1024, H=16 heads (hd=64), fp32 in/out.
    qkv = x @ in_proj_w.T + in_proj_b ; causal MHA ; out proj ; y = LN(x + attn_out)

Sharding (zero cross-core communication, 8 NeuronCores):
    core c -> batch b = c % 4, query-group g = c // 4.
    Causal zig-zag balance: g=0 owns query blocks [0:512) and [1536:2048),
    g=1 owns [512:1536). Every core computes full K/V for its batch,
    attention only for its own queries, then out-proj + residual +
    LayerNorm for its queries. Outputs are disjoint row sets.

v4 (v1 829us -> v2 462us -> v4): all-bf16 matmul datapath (fp8 measured
over the 2e-2 max-norm gate), plus:
  * Attention starts early: K^T is computed f-block-contiguous (one
    weight-slice DMA per block, all 16 k-tiles), and only f-block 0
    plus the V tiles qt0 needs are computed up front. The remaining
    K^T blocks, V tiles and the qt1 Q-projection are emitted as
    per-head-pair fillers inside attention(qt0) -- f-block hp+1 lands
    in head-pair hp's slot, just in time.
  * out-projection(qt0) chunks fill attention(qt1); out-proj(qt1) is
    emitted before both LayerNorms so its matmuls run during the LN
    DVE chains.
  * Softmax normalize copies the ctx PSUM accumulator to SBUF in one
    op, releasing the PSUM bank ~2us earlier per head pair.
  * Causal mask applied to both heads in one DVE op via a duplicated
    [128, 2, 896] mask tile.
  * exp merged: one ACTIVATE per (head-pair, k-tile) over [128, 2, 512]
    PSUM; in-proj V bias folded into the out-proj bias on the host.
"""
import sys

if "/opt/trn_rl_repo" not in sys.path:
    sys.path.insert(0, "/opt/trn_rl_repo")

import numpy as np

B, S, D, H, HD = 4, 2048, 1024, 16, 64
P = 128
QT = 512                      # queries per q-tile (matmul free dim)
NQ = 1024                     # queries per core
NKT = S // P                  # 16 k-tiles per batch
DK = D // P                   # 8 contraction tiles over D
NPLAIN = {0: (0, 12), 1: (4, 8)}   # group -> per-q-tile plain (unmasked) k-tiles
QBLOCK = {0: (0, 3), 1: (1, 2)}    # group -> 512-col x-block per q-tile

_cache = {}


def _build():
    import concourse.mybir as mybir
    import concourse.tile as tile
    from concourse import bacc
    from concourse.bass import ts
    from concourse.alu_op_type import AluOpType

    f32 = mybir.dt.float32
    bf16 = mybir.dt.bfloat16
    AF = mybir.ActivationFunctionType

    nc = bacc.Bacc("TRN2", target_bir_lowering=False, debug=False, num_devices=8)

    xkv = nc.dram_tensor("xkv", [D, S], bf16, kind="ExternalInput").ap()
    xrd = nc.dram_tensor("xrd", [D, S], f32, kind="ExternalInput").ap()
    wqd = nc.dram_tensor("wqd", [D, D], bf16, kind="ExternalInput").ap()
    wkd = nc.dram_tensor("wkd", [D, D], bf16, kind="ExternalInput").ap()
    wvd = nc.dram_tensor("wvd", [D, D], bf16, kind="ExternalInput").ap()
    wod = nc.dram_tensor("wod", [D, D], bf16, kind="ExternalInput").ap()
    maskd = nc.dram_tensor("maskd", [P, 896], bf16, kind="ExternalInput").ap()
    bqd = nc.dram_tensor("bqd", [D], f32, kind="ExternalInput").ap()
    bkd = nc.dram_tensor("bkd", [D], f32, kind="ExternalInput").ap()
    bod = nc.dram_tensor("bod", [D], f32, kind="ExternalInput").ap()
    gamd = nc.dram_tensor("gamd", [D], f32, kind="ExternalInput").ap()
    betd = nc.dram_tensor("betd", [D], f32, kind="ExternalInput").ap()
    yt = nc.dram_tensor("yt", [D, NQ], f32, kind="ExternalOutput").ap()

    xkv_r = xkv.rearrange("(dk p) t -> p dk t", p=P)
    xr_r = xrd.rearrange("(ok p) t -> p ok t", p=P)
    wq_r = wqd.rearrange("(dk p) (f c) -> p dk f c", p=P, c=P)
    wk_r = wkd.rearrange("(dk p) (f c) -> p dk f c", p=P, c=P)
    wv_r = wvd.rearrange("(dk p) (g c) -> p dk g c", p=P, c=512)
    wo_r = wod.rearrange("(dk p) (o c) -> p dk o c", p=P, c=P)

    with tile.TileContext(nc) as tc:
        with tc.tile_pool(name="pers", bufs=1) as pers:
            kt = pers.tile([P, DK, S], bf16)           # K^T        32 KB/part
            v = pers.tile([P, NKT, H, HD + 1], bf16)   # V aug      33.25 KB
            msk2 = pers.tile([P, 2, 896], bf16)        # mask x2    3.5 KB
            bia = pers.tile([P, DK, 5], f32)           # bq bk bo gam bet
            ones128 = pers.tile([P, 1], bf16)
            eps_t = pers.tile([1, 1], f32)
            wo = pers.tile([P, DK, DK, P], bf16)       # out_w^T    16 KB
            qtl = [
                pers.tile([P, DK, QT], bf16, tag="qtl0", name="qtl0"),
                pers.tile([P, DK, QT], bf16, tag="qtl1", name="qtl1"),
            ]
            # per-f-block ctx tiles so out-proj matmuls only depend on the
            # head pairs they actually read (tile-granular deps otherwise
            # serialize out-proj behind the last head's normalize)
            ctx = [
                [pers.tile([P, QT], bf16, tag=f"ctx{qt}_{ft}",
                           name=f"ctx{qt}_{ft}") for ft in range(DK)]
                for qt in range(2)
            ]

            nc.vector.memset(eps_t[:], 1e-5)
            nc.vector.memset(ones128[:], 1.0)
            nc.vector.memset(v[:, :, :, HD], 1.0)

            def bq_(f): return bia[:, f, 0:1]
            def bk_(f): return bia[:, f, 1:2]
            def bo_(f): return bia[:, f, 2:3]
            def gam_(f): return bia[:, f, 3:4]
            def bet_(f): return bia[:, f, 4:5]

            def emit_group(g):
                npl = NPLAIN[g]
                blocks = QBLOCK[g]

                with tc.tile_pool(name=f"pp{g}", bufs=2, space="PSUM") as pp:

                    def qchunk(qt, fh):
                        xs = x[:, :, ts(blocks[qt], QT)]
                        for f in range(4 * fh, 4 * fh + 4):
                            wsl = ph1.tile([P, DK, P], bf16, tag="wsl",
                                           bufs=3, name="wsl")
                            nc.sync.dma_start(wsl[:], wq_r[:, :, f, :])
                            ps = pp.tile([P, QT], f32, tag="pp", name="psq")
                            for dk in range(DK):
                                nc.tensor.matmul(
                                    ps[:], wsl[:, dk, :], xs[:, dk, :],
                                    start=(dk == 0), stop=(dk == DK - 1),
                                )
                            nc.vector.tensor_scalar_add(
                                qtl[qt][:, f, :], ps[:], bq_(f))

                    def achunk(f):
                        # K^T feature-block f for ALL 16 k-tiles
                        wsl = ph1.tile([P, DK, P], bf16, tag="wsl",
                                       bufs=3, name="wslk")
                        nc.sync.dma_start(wsl[:], wk_r[:, :, f, :])
                        for t in range(S // QT):
                            ps = pp.tile([P, QT], f32, tag="pp", name="psk")
                            for dk in range(DK):
                                nc.tensor.matmul(
                                    ps[:], wsl[:, dk, :], x[:, dk, ts(t, QT)],
                                    start=(dk == 0), stop=(dk == DK - 1),
                                )
                            nc.vector.tensor_scalar_add(
                                kt[:, f, ts(t, QT)], ps[:], bk_(f))

                    def bchunk(t):
                        for fg in range(2):
                            ps = pp.tile([P, 8, HD], f32, tag="pp", name="psv")
                            for dk in range(DK):
                                nc.tensor.matmul(
                                    ps[:], x[:, dk, ts(t, P)], wvs[:, dk, fg, :],
                                    start=(dk == 0), stop=(dk == DK - 1),
                                )
                            nc.scalar.copy(v[:, t, 8 * fg:8 * fg + 8, 0:HD], ps[:])

                    def attn(qt, fillers_by_hp):
                        nk = npl[qt] + 4
                        qtile = qtl[qt]
                        with (
                            tc.tile_pool(name=f"sep{g}{qt}", bufs=4) as sep,
                            tc.tile_pool(name=f"scr{g}{qt}", bufs=1) as scr,
                            tc.tile_pool(name=f"sps{g}{qt}", bufs=2, space="PSUM") as s_ps,
                            tc.tile_pool(name=f"cps{g}{qt}", bufs=1, space="PSUM") as c_ps,
                        ):
                            for hp in range(H // 2):
                                cp0 = c_ps.tile([HD + 1, QT], f32, tag="c0", name="cp0")
                                cp1 = c_ps.tile([HD + 1, QT], f32, tag="c1", name="cp1")
                                for i in range(nk):
                                    # queries below the diagonal band see
                                    # every key of this k-tile masked; skip
                                    # those columns in S, exp, mask and ctx
                                    lo = max(0, P * (i - npl[qt]))
                                    sp = s_ps.tile([P, 2, QT], f32, tag="s", name="sp")
                                    se = sep.tile([P, 2, QT], bf16, tag="se", name="se")
                                    nc.tensor.matmul(
                                        sp[:, 0, lo:QT], kt[0:HD, hp, ts(i, P)],
                                        qtile[0:HD, hp, lo:QT],
                                        start=True, stop=True,
                                    )
                                    nc.tensor.matmul(
                                        sp[:, 1, lo:QT], kt[HD:P, hp, ts(i, P)],
                                        qtile[HD:P, hp, lo:QT],
                                        start=True, stop=True,
                                    )
                                    nc.scalar.activation(
                                        se[:, :, lo:QT], sp[:, :, lo:QT],
                                        AF.Exp, scale=0.125)
                                    if i >= npl[qt]:
                                        nc.vector.tensor_mul(
                                            se[:, :, lo:QT], se[:, :, lo:QT],
                                            msk2[:, :, 384:384 + QT - lo])
                                    nc.tensor.matmul(
                                        cp0[:, lo:QT], v[:, i, 2 * hp, :],
                                        se[:, 0, lo:QT],
                                        start=(i == 0), stop=(i == nk - 1),
                                    )
                                    nc.tensor.matmul(
                                        cp1[:, lo:QT], v[:, i, 2 * hp + 1, :],
                                        se[:, 1, lo:QT],
                                        start=(i == 0), stop=(i == nk - 1),
                                    )
                                for j, cp in ((0, cp0), (1, cp1)):
                                    h = 2 * hp + j
                                    po, ft = HD * (h % 2), h // 2
                                    # one copy frees the PSUM bank for the
                                    # next head pair; normalize from SBUF
                                    cr = scr.tile([HD + 1, QT], bf16, tag="cr",
                                                  bufs=2, name="cr")
                                    nc.vector.tensor_copy(cr[:], cp[:])
                                    den = scr.tile([1, QT], f32, tag="den")
                                    nc.vector.tensor_copy(den[:], cr[HD:HD + 1, :])
                                    rec = scr.tile([1, QT], f32, tag="rec")
                                    rscr = scr.tile([1, QT], f32, tag="rscr")
                                    nc.vector.reciprocal_approx_accurate(
                                        rec[:], den[:], rscr[:])
                                    bc = scr.tile([HD, QT], f32, tag="bc", bufs=2)
                                    nc.gpsimd.partition_broadcast(bc[:], rec[:])
                                    nc.vector.tensor_mul(
                                        ctx[qt][ft][po:po + HD, :], cr[0:HD, :], bc[:])
                                for fill in fillers_by_hp[hp]:
                                    fill()

                    def outchunk(qt, o):
                        ps = pp.tile([P, QT], f32, tag="pp", name="pso")
                        for dk in range(DK):
                            nc.tensor.matmul(
                                ps[:], wo[:, dk, o, :], ctx[qt][dk][:, :],
                                start=(dk == 0), stop=(dk == DK - 1),
                            )
                        xr = ph2.tile([P, QT], f32, tag="xr", bufs=3, name="xr")
                        nc.sync.dma_start(
                            xr[:], xr_r[:, o, ts(blocks[qt], QT)])
                        nc.vector.scalar_tensor_tensor(
                            yts[qt][:, o, :], ps[:], bo_(o), xr[:],
                            AluOpType.add, AluOpType.add,
                        )

                    def stats_chunk(qt, o, st):
                        y = yts[qt]
                        mu_ps, ms_ps = st
                        nc.tensor.matmul(
                            mu_ps[:], ones128[:], y[:, o, :],
                            start=(o == 0), stop=(o == DK - 1))
                        ysq = ph2.tile([P, QT], bf16, tag="ysq", bufs=2, name="ysq")
                        nc.vector.tensor_mul(ysq[:], y[:, o, :], y[:, o, :])
                        nc.tensor.matmul(
                            ms_ps[:], ones128[:], ysq[:],
                            start=(o == 0), stop=(o == DK - 1))

                    def ln_chain(qt, st):
                        mu_ps, ms_ps = st
                        mu = ph2.tile([1, QT], f32, tag=f"mu{qt}", name="mu")
                        nc.scalar.mul(mu[:], mu_ps[:], 1.0 / D)
                        ms = ph2.tile([1, QT], f32, tag=f"ms{qt}", name="ms")
                        nc.scalar.mul(ms[:], ms_ps[:], 1.0 / D)
                        tmp = ph2.tile([1, QT], f32, tag="ltmp", bufs=2, name="tmp")
                        nc.vector.tensor_mul(tmp[:], mu[:], mu[:])
                        nc.vector.tensor_sub(ms[:], ms[:], tmp[:])  # var
                        sd = ph2.tile([1, QT], f32, tag="ltmp", bufs=2, name="sd")
                        nc.scalar.activation(sd[:], ms[:], AF.Sqrt, bias=eps_t[:])
                        rstd = ph2.tile([1, QT], f32, tag=f"rstd{qt}", name="rstd")
                        rsc = ph2.tile([1, QT], f32, tag="ltmp", bufs=2, name="rsc")
                        nc.vector.reciprocal_approx_accurate(rstd[:], sd[:], rsc[:])
                        mu_bc = ph2.tile([P, QT], f32, tag=f"mu_bc{qt}", name="mu_bc")
                        nc.gpsimd.partition_broadcast(mu_bc[:], mu[:])
                        rs_bc = ph2.tile([P, QT], f32, tag=f"rs_bc{qt}", name="rs_bc")
                        nc.gpsimd.partition_broadcast(rs_bc[:], rstd[:])
                        return mu_bc, rs_bc

                    def ln_final(qt, o, mu_bc, rs_bc):
                        t1 = ph2.tile([P, QT], bf16, tag="t1", bufs=2, name="t1")
                        nc.vector.tensor_sub(t1[:], yts[qt][:, o, :], mu_bc[:])
                        nc.vector.tensor_mul(t1[:], t1[:], rs_bc[:])
                        yo = ph2.tile([P, QT], f32, tag="yo", bufs=2, name="yo")
                        nc.vector.tensor_scalar(
                            yo[:], t1[:], gam_(o), bet_(o),
                            AluOpType.mult, AluOpType.add,
                        )
                        nc.sync.dma_start(yt[ts(o, P), ts(qt, QT)], yo[:])

                    # ---- phase 1 ------------------------------------------
                    nk0 = npl[0] + 4
                    with tc.tile_pool(name=f"ph1_{g}", bufs=1) as ph1:
                        x = ph1.tile([P, DK, S], bf16, name="x")
                        wvs = ph1.tile([P, DK, 2, 512], bf16, name="wvs")
                        # x block for qproj(0) first, split per dk-block so
                        # the first matmul starts after 128KB, not 1MB;
                        # small tables next; wo (first needed ~200us in) last
                        b0 = blocks[0]
                        for dk in range(DK):
                            nc.sync.dma_start(
                                x[:, dk, ts(b0, QT)], xkv_r[:, dk, ts(b0, QT)])
                        for j, src in enumerate((bqd, bkd, bod, gamd, betd)):
                            nc.sync.dma_start(
                                bia[:, :, j], src.rearrange("(f p) -> p f", p=P))
                        for t in range(S // QT):
                            if t != b0:
                                for dh in range(2):
                                    nc.sync.dma_start(
                                        x[:, 4 * dh:4 * dh + 4, ts(t, QT)],
                                        xkv_r[:, 4 * dh:4 * dh + 4, ts(t, QT)])
                        nc.sync.dma_start(msk2[:, 0, :], maskd[:])
                        nc.sync.dma_start(msk2[:, 1, :], maskd[:])
                        nc.sync.dma_start(wvs[:], wv_r)
                        nc.sync.dma_start(wo[:], wo_r)

                        qchunk(0, 0)
                        qchunk(0, 1)
                        achunk(0)
                        for t in range(nk0):
                            bchunk(t)

                        # per-head-pair fillers: K^T block hp+1 arrives just
                        # in time; V tail and qt1 Q-proj spread across slots
                        rest = [lambda t=t: bchunk(t) for t in range(nk0, NKT)]
                        rest += [lambda: qchunk(1, 0), lambda: qchunk(1, 1)]
                        fb = [[] for _ in range(8)]
                        for hp in range(7):
                            fb[hp].append(lambda f=hp + 1: achunk(f))
                        for k, r in enumerate(rest):
                            fb[k * 8 // len(rest)].append(r)
                        attn(0, fb)

                    # ---- phase 2 ------------------------------------------
                    with tc.tile_pool(name=f"ph2_{g}", bufs=1) as ph2:
                        yts = [
                            ph2.tile([P, DK, QT], bf16, tag="y0", name="y0"),
                            ph2.tile([P, DK, QT], bf16, tag="y1", name="y1"),
                        ]
                        # out-proj(qt0) fillers early (they only need ctx0);
                        # the last slot computes qt0's LN statistics in the
                        # pp PSUM ring, filling the final head's bubble
                        st0 = []

                        def stats0():
                            st0.append((
                                pp.tile([1, QT], f32, tag="pp", name="mu0p"),
                                pp.tile([1, QT], f32, tag="pp", name="ms0p")))
                            for o in range(DK):
                                stats_chunk(0, o, st0[0])

                        fb = [[] for _ in range(8)]
                        for o in range(DK):
                            fb[min(o, 6)].append(lambda o=o: outchunk(0, o))
                        fb[7].append(stats0)
                        attn(1, fb)
                        # tail: ln0 chain + finals overlap out-proj(qt1)
                        # and qt1 stats; only the ln1 chain is serial
                        with tc.tile_pool(name=f"st_{g}", bufs=1,
                                          space="PSUM") as stp:
                            st1 = (stp.tile([1, QT], f32, tag="mu1", name="mu1p"),
                                   stp.tile([1, QT], f32, tag="ms1", name="ms1p"))
                            bc0 = ln_chain(0, st0[0])
                            for o in range(DK):
                                outchunk(1, o)
                                ln_final(0, o, *bc0)
                                stats_chunk(1, o, st1)
                            bc1 = ln_chain(1, st1)
                            for o in range(DK):
                                ln_final(1, o, *bc1)

            pid = nc.partition_id()
            with tc.If(pid < 4) as cmp:
                emit_group(0)
            with cmp.Else():
                emit_group(1)
    nc.compile()
    return nc


def _get_nc():
    if "nc" not in _cache:
        _cache["nc"] = _build()
    return _cache["nc"]


def _prep(x, in_proj_w, in_proj_b, out_w, out_b, gamma, beta):
    import ml_dtypes

    bf16 = ml_dtypes.bfloat16
    x = np.asarray(x, np.float32)
    w = np.asarray(in_proj_w, np.float32)
    wq = np.ascontiguousarray(w[0:D].T.astype(bf16))
    wk = np.ascontiguousarray(w[D:2 * D].T.astype(bf16))
    wv = np.ascontiguousarray(w[2 * D:3 * D].T.astype(bf16))
    wo = np.asarray(out_w, np.float32)
    wot = np.ascontiguousarray(wo.T.astype(bf16))
    bqkv = np.asarray(in_proj_b, np.float32)
    # fold the V bias through the out projection (softmax weights sum to 1)
    bo_eff = (np.asarray(out_b, np.float32)
              + wo @ bqkv[2 * D:3 * D]).astype(np.float32)
    gam = np.asarray(gamma, np.float32)
    bet = np.asarray(beta, np.float32)
    ku = np.arange(P)[:, None] <= (np.arange(896)[None, :] - 384)
    maskd = ku.astype(bf16)
    qcols = {
        0: np.r_[0:QT, 3 * QT:4 * QT],
        1: np.r_[QT:3 * QT],
    }
    in_maps = []
    for c in range(8):
        b = c % 4
        xt = np.ascontiguousarray(x[b].T)
        in_maps.append({
            "xkv": np.ascontiguousarray(xt.astype(bf16)),
            "xrd": xt,
            "wqd": wq, "wkd": wk, "wvd": wv, "wod": wot,
            "maskd": maskd,
            "bqd": bqkv[0:D], "bkd": bqkv[D:2 * D],
            "bod": bo_eff, "gamd": gam, "betd": bet,
        })
    return in_maps, qcols


def _run(in_maps, trace=False, **kw):
    from concourse.bass_utils import run_bass_kernel_spmd

    return run_bass_kernel_spmd(_get_nc(), in_maps, list(range(8)), trace=trace, **kw)


def kernel(x, in_proj_w, in_proj_b, out_w, out_b, gamma, beta):
    in_maps, qcols = _prep(x, in_proj_w, in_proj_b, out_w, out_b, gamma, beta)
    res = _run(in_maps)
    out = np.empty((B, S, D), np.float32)
    for c in range(8):
        out[c % 4, qcols[c // 4]] = res.results[c]["yt"].T
    return out
